# revision 30
# baseline (speedup 1.0000x reference)
"""Bipartite GNN attention kernel for Trainium2, SPMD across 8 NeuronCores.

Math (per reference):
  u = user @ W_u.T + b_u ; v = item @ W_v.T + b_v
  learn_user = softmax((u @ v.T) * UV_adj * scale, axis=1) @ v + u
  learn_item = softmax((v @ u.T) * VU_adj * scale, axis=1) @ u + v

Sharding: core i owns rows [i*1024, (i+1)*1024) of BOTH outputs; no
collectives (the contracted-side projection is replicated).

v4 design (fp8 DoubleRow, deep-pipelined):
- All big matmuls (scores, aggregation, denominator, projections) run in
  fp8e4 with perf_mode=DoubleRow (2 k-chunks per instruction, ~1.5x PE).
- Feature matrices are projected twice: fT [h, N] (feature-major, biased,
  used as score lhsT) and vrow [N, h] (row-major, UNbiased, used as
  aggregation rhs). The missing bias in vrow cancels through softmax:
  P@(v + 1 b^T)/rsum = P@vrow/rsum + b^T, so b_feat is folded into the
  residual qrow instead. This removes all per-block PE transposes.
- Per-core inputs are column-ROLLED so this core's rows are columns
  [0:RB) of both feature matrices; the score rhs (qTb) is then just
  fT_other[:, :, 0:RB] - no separate query projection.
- exp uses bias -ln(32): softmax is shift-invariant, masked entries
  become exactly 1/32 (fp8-exact), max value ~5 stays far below fp8e4
  max 240.
- Residual path stays accurate: qrow = f32r projection of the f32 query
  rows -> bf16, + (b_q + b_feat) broadcast row.
- v4 scheduling: fp8 projections run FIRST (first matmul only needs
  ~0.5MB of DMA, not 3.5MB); qrow streams per-column-chunk; PSUM is
  organized as 8 one-bank tiles so projections get a 7-deep ring; the
  attention epilogue + final aggregation of each row-block are deferred
  into the next block's score loop so the PE never drains at block
  boundaries.
"""

import sys

sys.path.insert(0, "/opt/trn_rl_repo")

import ml_dtypes
import numpy as np

import concourse.bacc as bacc
import concourse.bass as bass
import concourse.mybir as mybir
import concourse.tile as tile
from concourse.bass_utils import run_bass_kernel_spmd

N = 8192          # users == items
H = 512           # hidden
NCORES = 8
RB = N // NCORES  # 1024 rows per core per direction
KH = H // 128     # 4 h-chunks
NB = N // 128     # 64 column chunks
NBP = NB // 2     # 32 column-pair chunks (DoubleRow)
NRB = RB // 512   # 2 r-blocks of 512
NJ = N // 512     # 16 512-col blocks for projection streaming
SCALE = float(1.0 / np.sqrt(np.float32(H)))
NLN32 = float(-np.log(32.0))

F32 = mybir.dt.float32
F32R = mybir.dt.float32r
BF16 = mybir.dt.bfloat16
FP8 = mybir.dt.float8e4
NP_FP8 = ml_dtypes.float8_e4m3
DR = mybir.MatmulPerfMode.DoubleRow


def build_nc():
    nc = bacc.Bacc("TRN2", target_bir_lowering=False, debug=False)

    featA = nc.declare_dram_parameter("featA", [H, N], FP8, isOutput=False)
    featB = nc.declare_dram_parameter("featB", [H, N], FP8, isOutput=False)
    qtA = nc.declare_dram_parameter("qtA", [4, 128, KH * 256], F32,
                                    isOutput=False)
    qtB = nc.declare_dram_parameter("qtB", [4, 128, KH * 256], F32,
                                    isOutput=False)
    maskA = nc.declare_dram_parameter("maskA", [NB, 128, RB], FP8, isOutput=False)
    maskB = nc.declare_dram_parameter("maskB", [NB, 128, RB], FP8, isOutput=False)
    WfA = nc.declare_dram_parameter("WfA", [128, KH, H], FP8, isOutput=False)
    WfB = nc.declare_dram_parameter("WfB", [128, KH, H], FP8, isOutput=False)
    WqA = nc.declare_dram_parameter("WqA", [H, H], F32, isOutput=False)
    WqB = nc.declare_dram_parameter("WqB", [H, H], F32, isOutput=False)
    bfA = nc.declare_dram_parameter("bfA", [128, KH], F32, isOutput=False)
    bfB = nc.declare_dram_parameter("bfB", [128, KH], F32, isOutput=False)
    brow = nc.declare_dram_parameter("brow", [128, H], F32, isOutput=False)
    out = nc.declare_dram_parameter("out", [2 * RB, H], F32, isOutput=True)

    with tile.TileContext(nc) as tc:
        with (
            tc.tile_pool(name="bigA", bufs=1) as bigA,
            tc.tile_pool(name="bigB", bufs=1) as bigB,
            tc.tile_pool(name="wts", bufs=1) as wts,
            tc.tile_pool(name="wtsB", bufs=1) as wtsB,
            tc.tile_pool(name="stream", bufs=6) as stream,
            tc.tile_pool(name="qc", bufs=2) as qcp,
            tc.tile_pool(name="mask", bufs=3) as maskp,
            tc.tile_pool(name="pf", bufs=3) as pfp,
            tc.tile_pool(name="pb", bufs=3) as pbp,
            tc.tile_pool(name="outs", bufs=1) as outsp,
            tc.tile_pool(name="small", bufs=1) as small,
            # 8 PSUM banks as single-bank tiles: ps_a holds the 4 agg
            # accumulators during attention (and joins the projection
            # ring before that), ps_b holds score tiles, ps_rs the
            # denominator.
            tc.tile_pool(name="ps_a", bufs=4, space="PSUM") as ps_a,
            tc.tile_pool(name="ps_b", bufs=3, space="PSUM") as ps_b,
            tc.tile_pool(name="ps_rs", bufs=1, space="PSUM") as ps_rs,
        ):
            ones2 = small.tile([128, 2, 16], FP8, tag="ones")
            nc.vector.memset(ones2[:], 1.0)
            nbias = small.tile([128, 1], F32, tag="nbias")
            nc.vector.memset(nbias[:], NLN32)
            brow_sb = small.tile([128, H], F32, tag="brow")
            nc.sync.dma_start(brow_sb[:], brow[:])
            bfA_sb = small.tile([128, KH], F32, tag="bfA")
            nc.sync.dma_start(bfA_sb[:], bfA[:])
            bfB_sb = small.tile([128, KH], F32, tag="bfB")
            nc.sync.dma_start(bfB_sb[:], bfB[:])

            # persistent per-direction tensors
            fT = {}
            vrow = {}
            qrow = {}
            for big_pool, d in ((bigA, "A"), (bigB, "B")):
                fT[d] = big_pool.tile([128, KH, N], FP8, tag=f"fT{d}",
                                      name=f"fT{d}")
                vrow[d] = big_pool.tile([128, NB, H], FP8, tag=f"vrow{d}",
                                        name=f"vrow{d}")
                qrow[d] = big_pool.tile([128, 2 * KH, H], BF16, tag=f"qrow{d}",
                                        name=f"qrow{d}")

            # 7-deep psum ring for the projection phase (attention pins
            # ps_a's 4 bufs as accumulators, projections may rotate
            # through everything).
            _ring = [0]

            def proj_ps():
                _ring[0] += 1
                pool = (ps_a, ps_b)[_ring[0] % 2]
                ps = pool.tile([128, 512], F32, tag="s",
                               name=f"ps{_ring[0]}")
                return ps

            # ---------------- phase 0a: fp8 projections ----------------
            def project_fp8(d, feat_dram, wf_dram, bias_f, side_tasks=()):
                side_tasks = list(side_tasks)
                wfp = wts.tile([128, KH, H], FP8, tag=f"wfp{d}",
                               name=f"wfp{d}")
                nc.sync.dma_start(wfp[:], wf_dram[:])
                for j in range(NJ):
                    ft_in = stream.tile([128, KH, 512], FP8, tag="ft",
                                        name=f"ft{d}{j}")
                    for k in range(KH):
                        nc.sync.dma_start(
                            ft_in[:, k, :],
                            feat_dram[k * 128:(k + 1) * 128,
                                      j * 512:(j + 1) * 512])
                    for m in range(KH):
                        ps = proj_ps()
                        for ko in range(2):
                            nc.tensor.matmul(
                                ps[:],
                                wfp[:, 2 * ko:2 * ko + 2, m * 128:(m + 1) * 128],
                                ft_in[:, 2 * ko:2 * ko + 2, :],
                                start=(ko == 0), stop=(ko == 1), perf_mode=DR)
                        if m % 2 == 0:
                            nc.vector.tensor_scalar(
                                out=fT[d][:, m, j * 512:(j + 1) * 512],
                                in0=ps[:], scalar1=bias_f[:, m:m + 1],
                                scalar2=None, op0=mybir.AluOpType.add)
                        else:
                            nc.scalar.add(
                                fT[d][:, m, j * 512:(j + 1) * 512], ps[:],
                                bias_f[:, m:m + 1])
                    for sub in range(4):
                        c = j * 4 + sub
                        ps = proj_ps()
                        for ko in range(2):
                            nc.tensor.matmul(
                                ps[:],
                                ft_in[:, 2 * ko:2 * ko + 2,
                                      sub * 128:(sub + 1) * 128],
                                wfp[:, 2 * ko:2 * ko + 2, :],
                                start=(ko == 0), stop=(ko == 1), perf_mode=DR)
                        if sub % 2 == 0:
                            nc.vector.tensor_copy(vrow[d][:, c, :], ps[:])
                        else:
                            nc.scalar.copy(vrow[d][:, c, :], ps[:])
                    # one interleaved residual-projection task per j-block:
                    # fills the DMA-gated moments of this stream with the
                    # qrow f32r matmuls, eliminating a separate qrow phase.
                    if side_tasks:
                        side_tasks.pop(0)()

            # ---------------- phase 0b: residual projections ----------------
            # wq for A lives in the wts pool; wq for B reuses the (now
            # idle) fp8 feature-stream pool slots, so its DMA can land
            # while qrow A computes.
            def load_wq(d, wq_dram, pool):
                wq = [pool.tile([128, H], F32R, tag=f"wq{k}", name=f"wq{d}{k}")
                      for k in range(KH)]
                for k in range(KH):
                    nc.sync.dma_start(
                        wq[k][:], wq_dram[k * 128:(k + 1) * 128, :].bitcast(F32R))
                return wq

            def qrow_task(d, qt_dram, c, state):
                def task():
                    c2, cc = divmod(c, 2)
                    if cc == 0:
                        qc = qcp.tile([128, KH, 256], F32R, tag="qt",
                                      name=f"qt{d}{c2}")
                        nc.sync.dma_start(
                            qc[:], qt_dram[c2].bitcast(F32R).rearrange(
                                "p (k c) -> p k c", k=KH))
                        state["qc"] = qc
                    qc = state["qc"]
                    wq = wq_sb[d]
                    ps = proj_ps()
                    for k in range(KH):
                        nc.tensor.matmul(
                            ps[:], qc[:, k, cc * 128:(cc + 1) * 128],
                            wq[k][:],
                            start=(k == 0), stop=(k == KH - 1))
                    nc.vector.tensor_tensor(
                        out=qrow[d][:, c, :], in0=ps[:], in1=brow_sb[:],
                        op=mybir.AluOpType.add)
                return task

            project_fp8("A", featA, WfA, bfA_sb)
            # wq DMAs hoisted here: they execute behind projection A's
            # feature stream, well before the qrow tasks need them.
            wq_sb = {"A": load_wq("A", WqA, wts), "B": load_wq("B", WqB, wtsB)}
            _qstate = {}
            project_fp8("B", featB, WfB, bfB_sb, side_tasks=(
                [qrow_task("A", qtA, c, _qstate) for c in range(2 * KH)]
                + [qrow_task("B", qtB, c, _qstate) for c in range(2 * KH)]))

            # ---------------- attention ----------------
            # Flat schedule over 4 row-blocks (A rb0, A rb1, B rb0, B rb1).
            # The final aggregation (pend) and the epilogue (prev) of each
            # block are emitted inside the NEXT block's score loop, so the
            # PE keeps streaming across boundaries.

            def emit_agg(agg4, rsum4, pbf2, bp, myvrow):
                for rs in range(4):
                    nc.tensor.matmul(
                        agg4[rs][:], pbf2[:, :, rs * 128:(rs + 1) * 128],
                        myvrow[:, 2 * bp:2 * bp + 2, :],
                        start=(bp == 0), stop=(bp == NBP - 1), perf_mode=DR)
                    # all 4 columns form ONE psum accumulation group (they
                    # share a 2KB zero region): start only on the very first
                    # matmul, stop only on the very last
                    nc.tensor.matmul(
                        rsum4[:, rs:rs + 1],
                        pbf2[:, :, rs * 128:(rs + 1) * 128],
                        ones2[:, :, 0:1],
                        start=(bp == 0 and rs == 0),
                        stop=(bp == NBP - 1 and rs == 3), perf_mode=DR)

            def emit_epilogue(d, rb, agg4, rsum4, out_base, final=False):
                recip = small.tile([128, 4], F32, tag="recip")
                nc.vector.reciprocal(recip[:], rsum4[:])
                o4 = outsp.tile([128, 4, H], F32, tag="o4", name=f"o{d}{rb}")
                for rs in range(4):
                    if rs % 2 == 0:
                        nc.vector.tensor_scalar(
                            out=o4[:, rs, :], in0=agg4[rs][:],
                            scalar1=recip[:, rs:rs + 1], scalar2=None,
                            op0=mybir.AluOpType.mult)
                    else:
                        nc.scalar.mul(o4[:, rs, :], agg4[rs][:],
                                      recip[:, rs:rs + 1])
                    # the +qrow adds touch only SBUF, so they go on the
                    # idle GpSimd: keeps the DVE/Scalar queues clear for
                    # the PSUM reads the next block's agg WAR-waits on.
                    # The FINAL epilogue is a serial tail (nothing overlaps
                    # it), and GpSimd adds are slow (~1.6us each) - split
                    # them with DVE there instead.
                    add_eng = nc.vector if (final and rs % 2 == 0) \
                        else nc.gpsimd
                    add_eng.tensor_tensor(
                        out=o4[:, rs, :], in0=o4[:, rs, :],
                        in1=qrow[d][:, rb * 4 + rs, :],
                        op=mybir.AluOpType.add)
                # one coalesced 1MB out-DMA per row-block instead of four
                # dispatches: row (rs*128+p) of the output block comes
                # from o4[p, rs, :]
                row0 = out_base + rb * 512
                nc.sync.dma_start(
                    out[row0:row0 + 512, :].rearrange("(r p) c -> p r c",
                                                      p=128),
                    o4[:])

            blocks = [(d, other, mask_dram, out_base, rb)
                      for (d, other, mask_dram, out_base) in
                      (("A", "B", maskA, 0), ("B", "A", maskB, RB))
                      for rb in range(NRB)]

            pendq = []   # (agg4, rsum4, pbf2, bp, vrow) awaiting aggregation
            prev = None  # (d, rb, agg4, rsum4, out_base) awaiting epilogue
            for bi, (d, other, mask_dram, out_base, rb) in enumerate(blocks):
                myfT = fT[d]
                qTb = fT[other]
                boundary = bi > 0
                agg4 = [ps_a.tile([128, 512], F32, tag="s",
                                  name=f"agg{d}{rb}_{rs}") for rs in range(4)]
                rsum4 = ps_rs.tile([128, 4], F32, tag="rs")
                for bp in range(NBP):
                    sps = []
                    for t in range(2):
                        b = 2 * bp + t
                        sp = ps_b.tile([128, 512], F32, tag="s")
                        for ko in range(2):
                            nc.tensor.matmul(
                                sp[:],
                                myfT[:, 2 * ko:2 * ko + 2,
                                     b * 128:(b + 1) * 128],
                                qTb[:, 2 * ko:2 * ko + 2,
                                    rb * 512:(rb + 1) * 512],
                                start=(ko == 0), stop=(ko == 1),
                                perf_mode=DR)
                        sps.append(sp)

                    # aggregate earlier pairs while DVE/Act chew on this
                    # one. At a block boundary: flush the old block's last
                    # aggregation + epilogue at bp0, then hold this
                    # block's first aggregations until bp2 so the new
                    # PSUM accumulation group doesn't wait on the old
                    # epilogue's reads.
                    if pendq:
                        emit_agg(*pendq.pop(0))
                    if prev is not None:
                        emit_epilogue(*prev)
                        prev = None
                    mt = maskp.tile([128, 2, 512], FP8, tag="mk")
                    nc.sync.dma_start(
                        mt[:],
                        mask_dram[2 * bp:2 * bp + 2, :,
                                  rb * 512:(rb + 1) * 512].rearrange(
                                      "t p c -> p t c"))
                    pbf2 = pbp.tile([128, 2, 512], FP8, tag="pbf")
                    for t in range(2):
                        # gpsimd cannot read PSUM; both mults go on DVE
                        p32 = pfp.tile([128, 512], F32, tag="p32")
                        nc.vector.tensor_tensor(
                            out=p32[:], in0=sps[t][:], in1=mt[:, t, :],
                            op=mybir.AluOpType.mult)
                        nc.scalar.activation(
                            pbf2[:, t, :], p32[:],
                            mybir.ActivationFunctionType.Exp,
                            bias=nbias[:], scale=SCALE)
                    pendq.append((agg4, rsum4, pbf2, bp, vrow[d]))
                prev = (d, rb, agg4, rsum4, out_base)
            while pendq:
                emit_agg(*pendq.pop(0))
            emit_epilogue(*prev, final=True)

    nc.compile()
    return nc


_NC_CACHE = None
TRACE = False
LAST_RESULT = None


def kernel(user, item, UV_adj, VU_adj, W_u, b_u, W_v, b_v):
    global _NC_CACHE, LAST_RESULT
    user = np.asarray(user, dtype=np.float32)
    item = np.asarray(item, dtype=np.float32)
    UV_adj = np.asarray(UV_adj, dtype=np.float32)
    VU_adj = np.asarray(VU_adj, dtype=np.float32)
    W_u = np.asarray(W_u, dtype=np.float32)
    W_v = np.asarray(W_v, dtype=np.float32)
    b_u = np.asarray(b_u, dtype=np.float32)
    b_v = np.asarray(b_v, dtype=np.float32)

    userT = np.ascontiguousarray(user.T)
    itemT = np.ascontiguousarray(item.T)
    userT8 = userT.astype(NP_FP8)
    itemT8 = itemT.astype(NP_FP8)
    UV8 = UV_adj.astype(NP_FP8)
    VU8 = np.ascontiguousarray(UV8.T)
    W_uT = np.ascontiguousarray(W_u.T)
    W_vT = np.ascontiguousarray(W_v.T)
    # [128, KH, H] fp8 weight layout for DoubleRow projections
    WfA_np = np.ascontiguousarray(
        W_vT.reshape(KH, 128, H).transpose(1, 0, 2).astype(NP_FP8))
    WfB_np = np.ascontiguousarray(
        W_uT.reshape(KH, 128, H).transpose(1, 0, 2).astype(NP_FP8))
    bfA_np = np.ascontiguousarray(b_v.reshape(KH, 128).T)
    bfB_np = np.ascontiguousarray(b_u.reshape(KH, 128).T)
    brow_np = np.ascontiguousarray(
        np.broadcast_to((b_u + b_v)[None, :], (128, H)))

    in_maps = []
    for i in range(NCORES):
        r = i * RB
        sl = slice(r, r + RB)
        in_maps.append({
            # feature matrices with this core's rows rolled to the front
            "featA": np.ascontiguousarray(np.roll(itemT8, -r, axis=1)),
            "featB": np.ascontiguousarray(np.roll(userT8, -r, axis=1)),
            "qtA": np.ascontiguousarray(
                userT[:, sl].reshape(KH, 128, 4, 256).transpose(2, 1, 0, 3)
                .reshape(4, 128, KH * 256)),
            "qtB": np.ascontiguousarray(
                itemT[:, sl].reshape(KH, 128, 4, 256).transpose(2, 1, 0, 3)
                .reshape(4, 128, KH * 256)),
            "maskA": np.ascontiguousarray(
                np.roll(VU8[:, sl], -r, axis=0)).reshape(NB, 128, RB),
            "maskB": np.ascontiguousarray(
                np.roll(UV8[:, sl], -r, axis=0)).reshape(NB, 128, RB),
            "WfA": WfA_np,
            "WfB": WfB_np,
            "WqA": W_uT,
            "WqB": W_vT,
            "bfA": bfA_np,
            "bfB": bfB_np,
            "brow": brow_np,
        })

    if _NC_CACHE is None:
        _NC_CACHE = build_nc()
    res = run_bass_kernel_spmd(_NC_CACHE, in_maps, core_ids=list(range(NCORES)),
                               trace=TRACE)
    LAST_RESULT = res
    results = res.results
    learn_user = np.concatenate([results[i]["out"][:RB] for i in range(NCORES)], 0)
    learn_item = np.concatenate([results[i]["out"][RB:] for i in range(NCORES)], 0)
    return (learn_user, learn_item)


if __name__ == "__main__":
    nc = build_nc()
    print("built ok")


# revision 31
# speedup vs baseline: 1.0246x; 1.0246x over previous
"""Bipartite GNN attention kernel for Trainium2, SPMD across 8 NeuronCores.

Math (per reference):
  u = user @ W_u.T + b_u ; v = item @ W_v.T + b_v
  learn_user = softmax((u @ v.T) * UV_adj * scale, axis=1) @ v + u
  learn_item = softmax((v @ u.T) * VU_adj * scale, axis=1) @ u + v

Sharding: core i owns rows [i*1024, (i+1)*1024) of BOTH outputs; no
collectives (the contracted-side projection is replicated).

v4 design (fp8 DoubleRow, deep-pipelined):
- All big matmuls (scores, aggregation, denominator, projections) run in
  fp8e4 with perf_mode=DoubleRow (2 k-chunks per instruction, ~1.5x PE).
- Feature matrices are projected twice: fT [h, N] (feature-major, biased,
  used as score lhsT) and vrow [N, h] (row-major, UNbiased, used as
  aggregation rhs). The missing bias in vrow cancels through softmax:
  P@(v + 1 b^T)/rsum = P@vrow/rsum + b^T, so b_feat is folded into the
  residual qrow instead. This removes all per-block PE transposes.
- Per-core inputs are column-ROLLED so this core's rows are columns
  [0:RB) of both feature matrices; the score rhs (qTb) is then just
  fT_other[:, :, 0:RB] - no separate query projection.
- exp uses bias -ln(32): softmax is shift-invariant, masked entries
  become exactly 1/32 (fp8-exact), max value ~5 stays far below fp8e4
  max 240.
- Residual path stays accurate: qrow = f32r projection of the f32 query
  rows -> bf16, + (b_q + b_feat) broadcast row.
- v4 scheduling: fp8 projections run FIRST (first matmul only needs
  ~0.5MB of DMA, not 3.5MB); qrow streams per-column-chunk; PSUM is
  organized as 8 one-bank tiles so projections get a 7-deep ring; the
  attention epilogue + final aggregation of each row-block are deferred
  into the next block's score loop so the PE never drains at block
  boundaries.
"""

import sys

sys.path.insert(0, "/opt/trn_rl_repo")

import ml_dtypes
import numpy as np

import concourse.bacc as bacc
import concourse.bass as bass
import concourse.mybir as mybir
import concourse.tile as tile
from concourse.bass_utils import run_bass_kernel_spmd

N = 8192          # users == items
H = 512           # hidden
NCORES = 8
RB = N // NCORES  # 1024 rows per core per direction
KH = H // 128     # 4 h-chunks
NB = N // 128     # 64 column chunks
NBP = NB // 2     # 32 column-pair chunks (DoubleRow)
NRB = RB // 512   # 2 r-blocks of 512
NJ = N // 512     # 16 512-col blocks for projection streaming
SCALE = float(1.0 / np.sqrt(np.float32(H)))
NLN32 = float(-np.log(32.0))

F32 = mybir.dt.float32
F32R = mybir.dt.float32r
BF16 = mybir.dt.bfloat16
FP8 = mybir.dt.float8e4
NP_FP8 = ml_dtypes.float8_e4m3
DR = mybir.MatmulPerfMode.DoubleRow


def build_nc():
    nc = bacc.Bacc("TRN2", target_bir_lowering=False, debug=False)

    featA = nc.declare_dram_parameter("featA", [H, N], FP8, isOutput=False)
    featB = nc.declare_dram_parameter("featB", [H, N], FP8, isOutput=False)
    qtA = nc.declare_dram_parameter("qtA", [4, 128, KH * 256], F32,
                                    isOutput=False)
    qtB = nc.declare_dram_parameter("qtB", [4, 128, KH * 256], F32,
                                    isOutput=False)
    maskA = nc.declare_dram_parameter("maskA", [NB, 128, RB], FP8, isOutput=False)
    maskB = nc.declare_dram_parameter("maskB", [NB, 128, RB], FP8, isOutput=False)
    WfA = nc.declare_dram_parameter("WfA", [128, KH, H], FP8, isOutput=False)
    WfB = nc.declare_dram_parameter("WfB", [128, KH, H], FP8, isOutput=False)
    WqA = nc.declare_dram_parameter("WqA", [H, H], F32, isOutput=False)
    WqB = nc.declare_dram_parameter("WqB", [H, H], F32, isOutput=False)
    bfA = nc.declare_dram_parameter("bfA", [128, KH], F32, isOutput=False)
    bfB = nc.declare_dram_parameter("bfB", [128, KH], F32, isOutput=False)
    brow = nc.declare_dram_parameter("brow", [128, H], F32, isOutput=False)
    out = nc.declare_dram_parameter("out", [2 * RB, H], F32, isOutput=True)

    with tile.TileContext(nc) as tc:
        with (
            tc.tile_pool(name="bigA", bufs=1) as bigA,
            tc.tile_pool(name="bigB", bufs=1) as bigB,
            tc.tile_pool(name="wts", bufs=1) as wts,
            tc.tile_pool(name="wtsB", bufs=1) as wtsB,
            tc.tile_pool(name="stream", bufs=6) as stream,
            tc.tile_pool(name="qc", bufs=2) as qcp,
            tc.tile_pool(name="mask", bufs=3) as maskp,
            tc.tile_pool(name="pf", bufs=3) as pfp,
            tc.tile_pool(name="pb", bufs=3) as pbp,
            tc.tile_pool(name="outs", bufs=1) as outsp,
            tc.tile_pool(name="small", bufs=1) as small,
            # 8 PSUM banks as single-bank tiles: ps_a holds the 4 agg
            # accumulators during attention (and joins the projection
            # ring before that), ps_b holds score tiles, ps_rs the
            # denominator.
            tc.tile_pool(name="ps_a", bufs=4, space="PSUM") as ps_a,
            tc.tile_pool(name="ps_b", bufs=3, space="PSUM") as ps_b,
            tc.tile_pool(name="ps_rs", bufs=1, space="PSUM") as ps_rs,
        ):
            ones2 = small.tile([128, 2, 16], FP8, tag="ones")
            nc.vector.memset(ones2[:], 1.0)
            nbias = small.tile([128, 1], F32, tag="nbias")
            nc.vector.memset(nbias[:], NLN32)
            brow_sb = small.tile([128, H], F32, tag="brow")
            nc.sync.dma_start(brow_sb[:], brow[:])
            bfA_sb = small.tile([128, KH], F32, tag="bfA")
            nc.sync.dma_start(bfA_sb[:], bfA[:])
            bfB_sb = small.tile([128, KH], F32, tag="bfB")
            nc.sync.dma_start(bfB_sb[:], bfB[:])

            # persistent per-direction tensors
            fT = {}
            vrow = {}
            qrow = {}
            for big_pool, d in ((bigA, "A"), (bigB, "B")):
                fT[d] = big_pool.tile([128, KH, N], FP8, tag=f"fT{d}",
                                      name=f"fT{d}")
                vrow[d] = big_pool.tile([128, NB, H], FP8, tag=f"vrow{d}",
                                        name=f"vrow{d}")
                qrow[d] = big_pool.tile([128, 2 * KH, H], BF16, tag=f"qrow{d}",
                                        name=f"qrow{d}")

            # 7-deep psum ring for the projection phase (attention pins
            # ps_a's 4 bufs as accumulators, projections may rotate
            # through everything).
            _ring = [0]

            def proj_ps():
                _ring[0] += 1
                pool = (ps_a, ps_b)[_ring[0] % 2]
                ps = pool.tile([128, 512], F32, tag="s",
                               name=f"ps{_ring[0]}")
                return ps

            # ---------------- phase 0a: fp8 projections ----------------
            def project_fp8(d, feat_dram, wf_dram, bias_f, side_tasks=()):
                side_tasks = list(side_tasks)
                wfp = wts.tile([128, KH, H], FP8, tag=f"wfp{d}",
                               name=f"wfp{d}")
                nc.sync.dma_start(wfp[:], wf_dram[:])
                for j in range(NJ):
                    ft_in = stream.tile([128, KH, 512], FP8, tag="ft",
                                        name=f"ft{d}{j}")
                    for k in range(KH):
                        nc.sync.dma_start(
                            ft_in[:, k, :],
                            feat_dram[k * 128:(k + 1) * 128,
                                      j * 512:(j + 1) * 512])
                    for m in range(KH):
                        ps = proj_ps()
                        for ko in range(2):
                            nc.tensor.matmul(
                                ps[:],
                                wfp[:, 2 * ko:2 * ko + 2, m * 128:(m + 1) * 128],
                                ft_in[:, 2 * ko:2 * ko + 2, :],
                                start=(ko == 0), stop=(ko == 1), perf_mode=DR)
                        if m % 2 == 0:
                            nc.vector.tensor_scalar(
                                out=fT[d][:, m, j * 512:(j + 1) * 512],
                                in0=ps[:], scalar1=bias_f[:, m:m + 1],
                                scalar2=None, op0=mybir.AluOpType.add)
                        else:
                            nc.scalar.add(
                                fT[d][:, m, j * 512:(j + 1) * 512], ps[:],
                                bias_f[:, m:m + 1])
                    for sub in range(4):
                        c = j * 4 + sub
                        ps = proj_ps()
                        for ko in range(2):
                            nc.tensor.matmul(
                                ps[:],
                                ft_in[:, 2 * ko:2 * ko + 2,
                                      sub * 128:(sub + 1) * 128],
                                wfp[:, 2 * ko:2 * ko + 2, :],
                                start=(ko == 0), stop=(ko == 1), perf_mode=DR)
                        if sub % 2 == 0:
                            nc.vector.tensor_copy(vrow[d][:, c, :], ps[:])
                        else:
                            nc.scalar.copy(vrow[d][:, c, :], ps[:])
                    # one interleaved residual-projection task per j-block:
                    # fills the DMA-gated moments of this stream with the
                    # qrow f32r matmuls, eliminating a separate qrow phase.
                    if side_tasks:
                        side_tasks.pop(0)()

            # ---------------- phase 0b: residual projections ----------------
            # wq for A lives in the wts pool; wq for B reuses the (now
            # idle) fp8 feature-stream pool slots, so its DMA can land
            # while qrow A computes.
            def load_wq(d, wq_dram, pool):
                wq = [pool.tile([128, H], F32R, tag=f"wq{k}", name=f"wq{d}{k}")
                      for k in range(KH)]
                for k in range(KH):
                    nc.sync.dma_start(
                        wq[k][:], wq_dram[k * 128:(k + 1) * 128, :].bitcast(F32R))
                return wq

            def qrow_task(d, qt_dram, c, state):
                def task():
                    c2, cc = divmod(c, 2)
                    if cc == 0:
                        qc = qcp.tile([128, KH, 256], F32R, tag="qt",
                                      name=f"qt{d}{c2}")
                        nc.sync.dma_start(
                            qc[:], qt_dram[c2].bitcast(F32R).rearrange(
                                "p (k c) -> p k c", k=KH))
                        state["qc"] = qc
                    qc = state["qc"]
                    wq = wq_sb[d]
                    ps = proj_ps()
                    for k in range(KH):
                        nc.tensor.matmul(
                            ps[:], qc[:, k, cc * 128:(cc + 1) * 128],
                            wq[k][:],
                            start=(k == 0), stop=(k == KH - 1))
                    nc.vector.tensor_tensor(
                        out=qrow[d][:, c, :], in0=ps[:], in1=brow_sb[:],
                        op=mybir.AluOpType.add)
                return task

            project_fp8("A", featA, WfA, bfA_sb)
            # wq DMAs hoisted here: they execute behind projection A's
            # feature stream, well before the qrow tasks need them.
            wq_sb = {"A": load_wq("A", WqA, wts), "B": load_wq("B", WqB, wtsB)}
            _qstate = {}
            project_fp8("B", featB, WfB, bfB_sb, side_tasks=(
                [qrow_task("A", qtA, c, _qstate) for c in range(2 * KH)]
                + [qrow_task("B", qtB, c, _qstate) for c in range(2 * KH)]))

            # ---------------- attention ----------------
            # Flat schedule over 4 row-blocks (A rb0, A rb1, B rb0, B rb1).
            # The final aggregation (pend) and the epilogue (prev) of each
            # block are emitted inside the NEXT block's score loop, so the
            # PE keeps streaming across boundaries.

            def emit_agg(agg4, rsum4, pbf2, bp, myvrow):
                for rs in range(4):
                    nc.tensor.matmul(
                        agg4[rs][:], pbf2[:, :, rs * 128:(rs + 1) * 128],
                        myvrow[:, 2 * bp:2 * bp + 2, :],
                        start=(bp == 0), stop=(bp == NBP - 1), perf_mode=DR)
                    # all 4 columns form ONE psum accumulation group (they
                    # share a 2KB zero region): start only on the very first
                    # matmul, stop only on the very last
                    nc.tensor.matmul(
                        rsum4[:, rs:rs + 1],
                        pbf2[:, :, rs * 128:(rs + 1) * 128],
                        ones2[:, :, 0:1],
                        start=(bp == 0 and rs == 0),
                        stop=(bp == NBP - 1 and rs == 3), perf_mode=DR)

            def emit_epilogue(d, rb, agg4, rsum4, out_base):
                recip = small.tile([128, 4], F32, tag="recip")
                nc.vector.reciprocal(recip[:], rsum4[:])
                for rs in range(4):
                    o_sb = outsp.tile([128, H], F32, tag=f"o{rs}",
                                      name=f"o{d}{rb}_{rs}")
                    if rs % 2 == 0:
                        nc.vector.tensor_scalar(
                            out=o_sb[:], in0=agg4[rs][:],
                            scalar1=recip[:, rs:rs + 1], scalar2=None,
                            op0=mybir.AluOpType.mult)
                    else:
                        nc.scalar.mul(o_sb[:], agg4[rs][:],
                                      recip[:, rs:rs + 1])
                    # the +qrow adds touch only SBUF, so they all go on
                    # the idle GpSimd: keeps the DVE/Scalar queues clear
                    # for the PSUM reads the next block's agg WAR-waits on
                    nc.gpsimd.tensor_tensor(
                        out=o_sb[:], in0=o_sb[:],
                        in1=qrow[d][:, rb * 4 + rs, :],
                        op=mybir.AluOpType.add)
                    row0 = out_base + rb * 512 + rs * 128
                    nc.sync.dma_start(out[row0:row0 + 128, :], o_sb[:])

            blocks = [(d, other, mask_dram, out_base, rb)
                      for (d, other, mask_dram, out_base) in
                      (("A", "B", maskA, 0), ("B", "A", maskB, RB))
                      for rb in range(NRB)]

            pendq = []   # (agg4, rsum4, pbf2, bp, vrow) awaiting aggregation
            prev = None  # (d, rb, agg4, rsum4, out_base) awaiting epilogue
            for bi, (d, other, mask_dram, out_base, rb) in enumerate(blocks):
                myfT = fT[d]
                qTb = fT[other]
                boundary = bi > 0
                agg4 = [ps_a.tile([128, 512], F32, tag="s",
                                  name=f"agg{d}{rb}_{rs}") for rs in range(4)]
                rsum4 = ps_rs.tile([128, 4], F32, tag="rs")
                for bp in range(NBP):
                    sps = []
                    for t in range(2):
                        b = 2 * bp + t
                        sp = ps_b.tile([128, 512], F32, tag="s")
                        for ko in range(2):
                            nc.tensor.matmul(
                                sp[:],
                                myfT[:, 2 * ko:2 * ko + 2,
                                     b * 128:(b + 1) * 128],
                                qTb[:, 2 * ko:2 * ko + 2,
                                    rb * 512:(rb + 1) * 512],
                                start=(ko == 0), stop=(ko == 1),
                                perf_mode=DR)
                        sps.append(sp)

                    # aggregate earlier pairs while DVE/Act chew on this
                    # one. At a block boundary: flush the old block's last
                    # aggregation + epilogue at bp0, then hold this
                    # block's first aggregations until bp2 so the new
                    # PSUM accumulation group doesn't wait on the old
                    # epilogue's reads.
                    if pendq:
                        emit_agg(*pendq.pop(0))
                    if prev is not None:
                        emit_epilogue(*prev)
                        prev = None
                    mt = maskp.tile([128, 2, 512], FP8, tag="mk")
                    nc.sync.dma_start(
                        mt[:],
                        mask_dram[2 * bp:2 * bp + 2, :,
                                  rb * 512:(rb + 1) * 512].rearrange(
                                      "t p c -> p t c"))
                    pbf2 = pbp.tile([128, 2, 512], FP8, tag="pbf")
                    for t in range(2):
                        # gpsimd cannot read PSUM; both mults go on DVE
                        p32 = pfp.tile([128, 512], F32, tag="p32")
                        nc.vector.tensor_tensor(
                            out=p32[:], in0=sps[t][:], in1=mt[:, t, :],
                            op=mybir.AluOpType.mult)
                        nc.scalar.activation(
                            pbf2[:, t, :], p32[:],
                            mybir.ActivationFunctionType.Exp,
                            bias=nbias[:], scale=SCALE)
                    pendq.append((agg4, rsum4, pbf2, bp, vrow[d]))
                prev = (d, rb, agg4, rsum4, out_base)
            while pendq:
                emit_agg(*pendq.pop(0))
            emit_epilogue(*prev)

    nc.compile()
    return nc


_NC_CACHE = None
TRACE = False
LAST_RESULT = None


def kernel(user, item, UV_adj, VU_adj, W_u, b_u, W_v, b_v):
    global _NC_CACHE, LAST_RESULT
    user = np.asarray(user, dtype=np.float32)
    item = np.asarray(item, dtype=np.float32)
    UV_adj = np.asarray(UV_adj, dtype=np.float32)
    VU_adj = np.asarray(VU_adj, dtype=np.float32)
    W_u = np.asarray(W_u, dtype=np.float32)
    W_v = np.asarray(W_v, dtype=np.float32)
    b_u = np.asarray(b_u, dtype=np.float32)
    b_v = np.asarray(b_v, dtype=np.float32)

    userT = np.ascontiguousarray(user.T)
    itemT = np.ascontiguousarray(item.T)
    userT8 = userT.astype(NP_FP8)
    itemT8 = itemT.astype(NP_FP8)
    UV8 = UV_adj.astype(NP_FP8)
    VU8 = np.ascontiguousarray(UV8.T)
    W_uT = np.ascontiguousarray(W_u.T)
    W_vT = np.ascontiguousarray(W_v.T)
    # [128, KH, H] fp8 weight layout for DoubleRow projections
    WfA_np = np.ascontiguousarray(
        W_vT.reshape(KH, 128, H).transpose(1, 0, 2).astype(NP_FP8))
    WfB_np = np.ascontiguousarray(
        W_uT.reshape(KH, 128, H).transpose(1, 0, 2).astype(NP_FP8))
    bfA_np = np.ascontiguousarray(b_v.reshape(KH, 128).T)
    bfB_np = np.ascontiguousarray(b_u.reshape(KH, 128).T)
    brow_np = np.ascontiguousarray(
        np.broadcast_to((b_u + b_v)[None, :], (128, H)))

    in_maps = []
    for i in range(NCORES):
        r = i * RB
        sl = slice(r, r + RB)
        in_maps.append({
            # feature matrices with this core's rows rolled to the front
            "featA": np.ascontiguousarray(np.roll(itemT8, -r, axis=1)),
            "featB": np.ascontiguousarray(np.roll(userT8, -r, axis=1)),
            "qtA": np.ascontiguousarray(
                userT[:, sl].reshape(KH, 128, 4, 256).transpose(2, 1, 0, 3)
                .reshape(4, 128, KH * 256)),
            "qtB": np.ascontiguousarray(
                itemT[:, sl].reshape(KH, 128, 4, 256).transpose(2, 1, 0, 3)
                .reshape(4, 128, KH * 256)),
            "maskA": np.ascontiguousarray(
                np.roll(VU8[:, sl], -r, axis=0)).reshape(NB, 128, RB),
            "maskB": np.ascontiguousarray(
                np.roll(UV8[:, sl], -r, axis=0)).reshape(NB, 128, RB),
            "WfA": WfA_np,
            "WfB": WfB_np,
            "WqA": W_uT,
            "WqB": W_vT,
            "bfA": bfA_np,
            "bfB": bfB_np,
            "brow": brow_np,
        })

    if _NC_CACHE is None:
        _NC_CACHE = build_nc()
    res = run_bass_kernel_spmd(_NC_CACHE, in_maps, core_ids=list(range(NCORES)),
                               trace=TRACE)
    LAST_RESULT = res
    results = res.results
    learn_user = np.concatenate([results[i]["out"][:RB] for i in range(NCORES)], 0)
    learn_item = np.concatenate([results[i]["out"][RB:] for i in range(NCORES)], 0)
    return (learn_user, learn_item)


if __name__ == "__main__":
    nc = build_nc()
    print("built ok")


# revision 33
# speedup vs baseline: 1.0320x; 1.0072x over previous
"""Bipartite GNN attention kernel for Trainium2, SPMD across 8 NeuronCores.

Math (per reference):
  u = user @ W_u.T + b_u ; v = item @ W_v.T + b_v
  learn_user = softmax((u @ v.T) * UV_adj * scale, axis=1) @ v + u
  learn_item = softmax((v @ u.T) * VU_adj * scale, axis=1) @ u + v

Sharding: core i owns rows [i*1024, (i+1)*1024) of BOTH outputs; no
collectives (the contracted-side projection is replicated).

v4 design (fp8 DoubleRow, deep-pipelined):
- All big matmuls (scores, aggregation, denominator, projections) run in
  fp8e4 with perf_mode=DoubleRow (2 k-chunks per instruction, ~1.5x PE).
- Feature matrices are projected twice: fT [h, N] (feature-major, biased,
  used as score lhsT) and vrow [N, h] (row-major, UNbiased, used as
  aggregation rhs). The missing bias in vrow cancels through softmax:
  P@(v + 1 b^T)/rsum = P@vrow/rsum + b^T, so b_feat is folded into the
  residual qrow instead. This removes all per-block PE transposes.
- Per-core inputs are column-ROLLED so this core's rows are columns
  [0:RB) of both feature matrices; the score rhs (qTb) is then just
  fT_other[:, :, 0:RB] - no separate query projection.
- exp uses bias -ln(32): softmax is shift-invariant, masked entries
  become exactly 1/32 (fp8-exact), max value ~5 stays far below fp8e4
  max 240.
- Residual path stays accurate: qrow = f32r projection of the f32 query
  rows -> bf16, + (b_q + b_feat) broadcast row.
- v4 scheduling: fp8 projections run FIRST (first matmul only needs
  ~0.5MB of DMA, not 3.5MB); qrow streams per-column-chunk; PSUM is
  organized as 8 one-bank tiles so projections get a 7-deep ring; the
  attention epilogue + final aggregation of each row-block are deferred
  into the next block's score loop so the PE never drains at block
  boundaries.
"""

import sys

sys.path.insert(0, "/opt/trn_rl_repo")

import ml_dtypes
import numpy as np

import concourse.bacc as bacc
import concourse.bass as bass
import concourse.mybir as mybir
import concourse.tile as tile
from concourse.bass_utils import run_bass_kernel_spmd

N = 8192          # users == items
H = 512           # hidden
NCORES = 8
RB = N // NCORES  # 1024 rows per core per direction
KH = H // 128     # 4 h-chunks
NB = N // 128     # 64 column chunks
NBP = NB // 2     # 32 column-pair chunks (DoubleRow)
NRB = RB // 512   # 2 r-blocks of 512
NJ = N // 512     # 16 512-col blocks for projection streaming
SCALE = float(1.0 / np.sqrt(np.float32(H)))
NLN32 = float(-np.log(32.0))

F32 = mybir.dt.float32
F32R = mybir.dt.float32r
BF16 = mybir.dt.bfloat16
FP8 = mybir.dt.float8e4
NP_FP8 = ml_dtypes.float8_e4m3
DR = mybir.MatmulPerfMode.DoubleRow


def build_nc():
    nc = bacc.Bacc("TRN2", target_bir_lowering=False, debug=False)

    featA = nc.declare_dram_parameter("featA", [H, N], FP8, isOutput=False)
    featB = nc.declare_dram_parameter("featB", [H, N], FP8, isOutput=False)
    qtA = nc.declare_dram_parameter("qtA", [4, 128, KH * 256], F32,
                                    isOutput=False)
    qtB = nc.declare_dram_parameter("qtB", [4, 128, KH * 256], F32,
                                    isOutput=False)
    maskA = nc.declare_dram_parameter("maskA", [NB, 128, RB], FP8, isOutput=False)
    maskB = nc.declare_dram_parameter("maskB", [NB, 128, RB], FP8, isOutput=False)
    WfA = nc.declare_dram_parameter("WfA", [128, KH, H], FP8, isOutput=False)
    WfB = nc.declare_dram_parameter("WfB", [128, KH, H], FP8, isOutput=False)
    WqA = nc.declare_dram_parameter("WqA", [H, H], F32, isOutput=False)
    WqB = nc.declare_dram_parameter("WqB", [H, H], F32, isOutput=False)
    bfA = nc.declare_dram_parameter("bfA", [128, KH], F32, isOutput=False)
    bfB = nc.declare_dram_parameter("bfB", [128, KH], F32, isOutput=False)
    brow = nc.declare_dram_parameter("brow", [128, H], F32, isOutput=False)
    out = nc.declare_dram_parameter("out", [2 * RB, H], F32, isOutput=True)

    with tile.TileContext(nc) as tc:
        with (
            tc.tile_pool(name="bigA", bufs=1) as bigA,
            tc.tile_pool(name="bigB", bufs=1) as bigB,
            tc.tile_pool(name="wts", bufs=1) as wts,
            tc.tile_pool(name="wtsB", bufs=1) as wtsB,
            tc.tile_pool(name="stream", bufs=6) as stream,
            tc.tile_pool(name="qc", bufs=2) as qcp,
            tc.tile_pool(name="mask", bufs=3) as maskp,
            tc.tile_pool(name="pf", bufs=3) as pfp,
            tc.tile_pool(name="pb", bufs=3) as pbp,
            tc.tile_pool(name="outs", bufs=1) as outsp,
            tc.tile_pool(name="small", bufs=1) as small,
            # 8 PSUM banks as single-bank tiles: ps_a holds the 4 agg
            # accumulators during attention (and joins the projection
            # ring before that), ps_b holds score tiles, ps_rs the
            # denominator.
            tc.tile_pool(name="ps_a", bufs=4, space="PSUM") as ps_a,
            tc.tile_pool(name="ps_b", bufs=3, space="PSUM") as ps_b,
            tc.tile_pool(name="ps_rs", bufs=1, space="PSUM") as ps_rs,
        ):
            ones2 = small.tile([128, 2, 16], FP8, tag="ones")
            nc.vector.memset(ones2[:], 1.0)
            nbias = small.tile([128, 1], F32, tag="nbias")
            nc.vector.memset(nbias[:], NLN32)
            brow_sb = small.tile([128, H], F32, tag="brow")
            nc.sync.dma_start(brow_sb[:], brow[:])
            bfA_sb = small.tile([128, KH], F32, tag="bfA")
            nc.sync.dma_start(bfA_sb[:], bfA[:])
            bfB_sb = small.tile([128, KH], F32, tag="bfB")
            nc.sync.dma_start(bfB_sb[:], bfB[:])

            # persistent per-direction tensors
            fT = {}
            vrow = {}
            qrow = {}
            for big_pool, d in ((bigA, "A"), (bigB, "B")):
                fT[d] = big_pool.tile([128, KH, N], FP8, tag=f"fT{d}",
                                      name=f"fT{d}")
                vrow[d] = big_pool.tile([128, NB, H], FP8, tag=f"vrow{d}",
                                        name=f"vrow{d}")
                qrow[d] = big_pool.tile([128, 2 * KH, H], BF16, tag=f"qrow{d}",
                                        name=f"qrow{d}")

            # 7-deep psum ring for the projection phase (attention pins
            # ps_a's 4 bufs as accumulators, projections may rotate
            # through everything).
            _ring = [0]

            def proj_ps():
                _ring[0] += 1
                pool = (ps_a, ps_b)[_ring[0] % 2]
                ps = pool.tile([128, 512], F32, tag="s",
                               name=f"ps{_ring[0]}")
                return ps

            # ---------------- phase 0a: fp8 projections ----------------
            def project_fp8(d, feat_dram, wf_dram, bias_f, side_tasks=()):
                side_tasks = list(side_tasks)
                wfp = wts.tile([128, KH, H], FP8, tag=f"wfp{d}",
                               name=f"wfp{d}")
                nc.sync.dma_start(wfp[:], wf_dram[:])
                for j in range(NJ):
                    ft_in = stream.tile([128, KH, 512], FP8, tag="ft",
                                        name=f"ft{d}{j}")
                    for k in range(KH):
                        nc.sync.dma_start(
                            ft_in[:, k, :],
                            feat_dram[k * 128:(k + 1) * 128,
                                      j * 512:(j + 1) * 512])
                    for m in range(KH):
                        ps = proj_ps()
                        for ko in range(2):
                            nc.tensor.matmul(
                                ps[:],
                                wfp[:, 2 * ko:2 * ko + 2, m * 128:(m + 1) * 128],
                                ft_in[:, 2 * ko:2 * ko + 2, :],
                                start=(ko == 0), stop=(ko == 1), perf_mode=DR)
                        if m % 2 == 0:
                            nc.vector.tensor_scalar(
                                out=fT[d][:, m, j * 512:(j + 1) * 512],
                                in0=ps[:], scalar1=bias_f[:, m:m + 1],
                                scalar2=None, op0=mybir.AluOpType.add)
                        else:
                            nc.scalar.add(
                                fT[d][:, m, j * 512:(j + 1) * 512], ps[:],
                                bias_f[:, m:m + 1])
                    for sub in range(4):
                        c = j * 4 + sub
                        ps = proj_ps()
                        for ko in range(2):
                            nc.tensor.matmul(
                                ps[:],
                                ft_in[:, 2 * ko:2 * ko + 2,
                                      sub * 128:(sub + 1) * 128],
                                wfp[:, 2 * ko:2 * ko + 2, :],
                                start=(ko == 0), stop=(ko == 1), perf_mode=DR)
                        if sub % 2 == 0:
                            nc.vector.tensor_copy(vrow[d][:, c, :], ps[:])
                        else:
                            nc.scalar.copy(vrow[d][:, c, :], ps[:])
                    # one interleaved residual-projection task per j-block:
                    # fills the DMA-gated moments of this stream with the
                    # qrow f32r matmuls, eliminating a separate qrow phase.
                    if side_tasks:
                        side_tasks.pop(0)()

            # ---------------- phase 0b: residual projections ----------------
            # wq for A lives in the wts pool; wq for B reuses the (now
            # idle) fp8 feature-stream pool slots, so its DMA can land
            # while qrow A computes.
            def load_wq(d, wq_dram, pool):
                wq = [pool.tile([128, H], F32R, tag=f"wq{k}", name=f"wq{d}{k}")
                      for k in range(KH)]
                for k in range(KH):
                    nc.sync.dma_start(
                        wq[k][:], wq_dram[k * 128:(k + 1) * 128, :].bitcast(F32R))
                return wq

            def qrow_task(d, qt_dram, c, state):
                def task():
                    c2, cc = divmod(c, 2)
                    if cc == 0:
                        qc = qcp.tile([128, KH, 256], F32R, tag="qt",
                                      name=f"qt{d}{c2}")
                        nc.sync.dma_start(
                            qc[:], qt_dram[c2].bitcast(F32R).rearrange(
                                "p (k c) -> p k c", k=KH))
                        state["qc"] = qc
                    qc = state["qc"]
                    wq = wq_sb[d]
                    ps = proj_ps()
                    for k in range(KH):
                        nc.tensor.matmul(
                            ps[:], qc[:, k, cc * 128:(cc + 1) * 128],
                            wq[k][:],
                            start=(k == 0), stop=(k == KH - 1))
                    nc.vector.tensor_tensor(
                        out=qrow[d][:, c, :], in0=ps[:], in1=brow_sb[:],
                        op=mybir.AluOpType.add)
                return task

            project_fp8("A", featA, WfA, bfA_sb)
            # wq DMAs hoisted here: they execute behind projection A's
            # feature stream, well before the qrow tasks need them.
            wq_sb = {"A": load_wq("A", WqA, wts), "B": load_wq("B", WqB, wtsB)}
            _qstate = {}
            project_fp8("B", featB, WfB, bfB_sb, side_tasks=(
                [qrow_task("A", qtA, c, _qstate) for c in range(2 * KH)]
                + [qrow_task("B", qtB, c, _qstate) for c in range(2 * KH)]))

            # ---------------- attention ----------------
            # Flat schedule over 4 row-blocks (A rb0, A rb1, B rb0, B rb1).
            # The final aggregation (pend) and the epilogue (prev) of each
            # block are emitted inside the NEXT block's score loop, so the
            # PE keeps streaming across boundaries.

            def emit_agg(agg4, rsum4, pbf2, bp, myvrow, rsum_first=False):
                if rsum_first:
                    # final flush only: the epilogue's recip waits on the
                    # rsum stop, so complete the denominator before the
                    # agg matmuls (steady-state order stays interleaved -
                    # reordering it regresses PE pacing).
                    for rs in range(4):
                        nc.tensor.matmul(
                            rsum4[:, rs:rs + 1],
                            pbf2[:, :, rs * 128:(rs + 1) * 128],
                            ones2[:, :, 0:1],
                            start=(bp == 0 and rs == 0),
                            stop=(bp == NBP - 1 and rs == 3), perf_mode=DR)
                    for rs in range(4):
                        nc.tensor.matmul(
                            agg4[rs][:], pbf2[:, :, rs * 128:(rs + 1) * 128],
                            myvrow[:, 2 * bp:2 * bp + 2, :],
                            start=(bp == 0), stop=(bp == NBP - 1),
                            perf_mode=DR)
                    return
                for rs in range(4):
                    nc.tensor.matmul(
                        agg4[rs][:], pbf2[:, :, rs * 128:(rs + 1) * 128],
                        myvrow[:, 2 * bp:2 * bp + 2, :],
                        start=(bp == 0), stop=(bp == NBP - 1), perf_mode=DR)
                    # all 4 columns form ONE psum accumulation group (they
                    # share a 2KB zero region): start only on the very first
                    # matmul, stop only on the very last
                    nc.tensor.matmul(
                        rsum4[:, rs:rs + 1],
                        pbf2[:, :, rs * 128:(rs + 1) * 128],
                        ones2[:, :, 0:1],
                        start=(bp == 0 and rs == 0),
                        stop=(bp == NBP - 1 and rs == 3), perf_mode=DR)

            def emit_epilogue(d, rb, agg4, rsum4, out_base, final=False):
                recip = small.tile([128, 4], F32, tag="recip")
                nc.vector.reciprocal(recip[:], rsum4[:])
                for rs in range(4):
                    o_sb = outsp.tile([128, H], F32, tag=f"o{rs}",
                                      name=f"o{d}{rb}_{rs}")
                    if rs % 2 == 0:
                        nc.vector.tensor_scalar(
                            out=o_sb[:], in0=agg4[rs][:],
                            scalar1=recip[:, rs:rs + 1], scalar2=None,
                            op0=mybir.AluOpType.mult)
                    else:
                        nc.scalar.mul(o_sb[:], agg4[rs][:],
                                      recip[:, rs:rs + 1])
                    # the +qrow adds touch only SBUF, so they go on the
                    # idle GpSimd (mid-kernel: keeps DVE/Scalar clear for
                    # the PSUM reads the next block's agg WAR-waits on);
                    # the FINAL epilogue is a serial tail, so split its
                    # adds with DVE (GpSimd adds are ~1.6us each).
                    add_eng = nc.vector if (final and rs % 2 == 0) \
                        else nc.gpsimd
                    add_eng.tensor_tensor(
                        out=o_sb[:], in0=o_sb[:],
                        in1=qrow[d][:, rb * 4 + rs, :],
                        op=mybir.AluOpType.add)
                    row0 = out_base + rb * 512 + rs * 128
                    # final epilogue: spread the four out-DMAs across four
                    # engine queues so the 256KB transfers run in parallel
                    # instead of serializing behind one dispatch queue -
                    # the end-of-kernel barrier waits on the last packet.
                    dma_eng = (nc.sync, nc.scalar, nc.gpsimd,
                               nc.scalar)[rs] if final else nc.sync
                    dma_eng.dma_start(out[row0:row0 + 128, :], o_sb[:])

            blocks = [(d, other, mask_dram, out_base, rb)
                      for (d, other, mask_dram, out_base) in
                      (("A", "B", maskA, 0), ("B", "A", maskB, RB))
                      for rb in range(NRB)]

            pendq = []   # (agg4, rsum4, pbf2, bp, vrow) awaiting aggregation
            prev = None  # (d, rb, agg4, rsum4, out_base) awaiting epilogue
            for bi, (d, other, mask_dram, out_base, rb) in enumerate(blocks):
                myfT = fT[d]
                qTb = fT[other]
                boundary = bi > 0
                agg4 = [ps_a.tile([128, 512], F32, tag="s",
                                  name=f"agg{d}{rb}_{rs}") for rs in range(4)]
                rsum4 = ps_rs.tile([128, 4], F32, tag="rs")
                for bp in range(NBP):
                    sps = []
                    for t in range(2):
                        b = 2 * bp + t
                        sp = ps_b.tile([128, 512], F32, tag="s")
                        for ko in range(2):
                            nc.tensor.matmul(
                                sp[:],
                                myfT[:, 2 * ko:2 * ko + 2,
                                     b * 128:(b + 1) * 128],
                                qTb[:, 2 * ko:2 * ko + 2,
                                    rb * 512:(rb + 1) * 512],
                                start=(ko == 0), stop=(ko == 1),
                                perf_mode=DR)
                        sps.append(sp)

                    # aggregate earlier pairs while DVE/Act chew on this
                    # one. At a block boundary: flush the old block's last
                    # aggregation + epilogue at bp0, then hold this
                    # block's first aggregations until bp2 so the new
                    # PSUM accumulation group doesn't wait on the old
                    # epilogue's reads.
                    if pendq:
                        emit_agg(*pendq.pop(0))
                    if prev is not None:
                        emit_epilogue(*prev)
                        prev = None
                    mt = maskp.tile([128, 2, 512], FP8, tag="mk")
                    nc.sync.dma_start(
                        mt[:],
                        mask_dram[2 * bp:2 * bp + 2, :,
                                  rb * 512:(rb + 1) * 512].rearrange(
                                      "t p c -> p t c"))
                    pbf2 = pbp.tile([128, 2, 512], FP8, tag="pbf")
                    for t in range(2):
                        # gpsimd cannot read PSUM; both mults go on DVE
                        p32 = pfp.tile([128, 512], F32, tag="p32")
                        nc.vector.tensor_tensor(
                            out=p32[:], in0=sps[t][:], in1=mt[:, t, :],
                            op=mybir.AluOpType.mult)
                        nc.scalar.activation(
                            pbf2[:, t, :], p32[:],
                            mybir.ActivationFunctionType.Exp,
                            bias=nbias[:], scale=SCALE)
                    pendq.append((agg4, rsum4, pbf2, bp, vrow[d]))
                prev = (d, rb, agg4, rsum4, out_base)
            while pendq:
                emit_agg(*pendq.pop(0), rsum_first=True)
            emit_epilogue(*prev, final=True)

    nc.compile()
    return nc


_NC_CACHE = None
TRACE = False
LAST_RESULT = None


def kernel(user, item, UV_adj, VU_adj, W_u, b_u, W_v, b_v):
    global _NC_CACHE, LAST_RESULT
    user = np.asarray(user, dtype=np.float32)
    item = np.asarray(item, dtype=np.float32)
    UV_adj = np.asarray(UV_adj, dtype=np.float32)
    VU_adj = np.asarray(VU_adj, dtype=np.float32)
    W_u = np.asarray(W_u, dtype=np.float32)
    W_v = np.asarray(W_v, dtype=np.float32)
    b_u = np.asarray(b_u, dtype=np.float32)
    b_v = np.asarray(b_v, dtype=np.float32)

    userT = np.ascontiguousarray(user.T)
    itemT = np.ascontiguousarray(item.T)
    userT8 = userT.astype(NP_FP8)
    itemT8 = itemT.astype(NP_FP8)
    UV8 = UV_adj.astype(NP_FP8)
    VU8 = np.ascontiguousarray(UV8.T)
    W_uT = np.ascontiguousarray(W_u.T)
    W_vT = np.ascontiguousarray(W_v.T)
    # [128, KH, H] fp8 weight layout for DoubleRow projections
    WfA_np = np.ascontiguousarray(
        W_vT.reshape(KH, 128, H).transpose(1, 0, 2).astype(NP_FP8))
    WfB_np = np.ascontiguousarray(
        W_uT.reshape(KH, 128, H).transpose(1, 0, 2).astype(NP_FP8))
    bfA_np = np.ascontiguousarray(b_v.reshape(KH, 128).T)
    bfB_np = np.ascontiguousarray(b_u.reshape(KH, 128).T)
    brow_np = np.ascontiguousarray(
        np.broadcast_to((b_u + b_v)[None, :], (128, H)))

    in_maps = []
    for i in range(NCORES):
        r = i * RB
        sl = slice(r, r + RB)
        in_maps.append({
            # feature matrices with this core's rows rolled to the front
            "featA": np.ascontiguousarray(np.roll(itemT8, -r, axis=1)),
            "featB": np.ascontiguousarray(np.roll(userT8, -r, axis=1)),
            "qtA": np.ascontiguousarray(
                userT[:, sl].reshape(KH, 128, 4, 256).transpose(2, 1, 0, 3)
                .reshape(4, 128, KH * 256)),
            "qtB": np.ascontiguousarray(
                itemT[:, sl].reshape(KH, 128, 4, 256).transpose(2, 1, 0, 3)
                .reshape(4, 128, KH * 256)),
            "maskA": np.ascontiguousarray(
                np.roll(VU8[:, sl], -r, axis=0)).reshape(NB, 128, RB),
            "maskB": np.ascontiguousarray(
                np.roll(UV8[:, sl], -r, axis=0)).reshape(NB, 128, RB),
            "WfA": WfA_np,
            "WfB": WfB_np,
            "WqA": W_uT,
            "WqB": W_vT,
            "bfA": bfA_np,
            "bfB": bfB_np,
            "brow": brow_np,
        })

    if _NC_CACHE is None:
        _NC_CACHE = build_nc()
    res = run_bass_kernel_spmd(_NC_CACHE, in_maps, core_ids=list(range(NCORES)),
                               trace=TRACE)
    LAST_RESULT = res
    results = res.results
    learn_user = np.concatenate([results[i]["out"][:RB] for i in range(NCORES)], 0)
    learn_item = np.concatenate([results[i]["out"][RB:] for i in range(NCORES)], 0)
    return (learn_user, learn_item)


if __name__ == "__main__":
    nc = build_nc()
    print("built ok")


# revision 34
# speedup vs baseline: 1.0362x; 1.0041x over previous
"""Bipartite GNN attention kernel for Trainium2, SPMD across 8 NeuronCores.

Math (per reference):
  u = user @ W_u.T + b_u ; v = item @ W_v.T + b_v
  learn_user = softmax((u @ v.T) * UV_adj * scale, axis=1) @ v + u
  learn_item = softmax((v @ u.T) * VU_adj * scale, axis=1) @ u + v

Sharding: core i owns rows [i*1024, (i+1)*1024) of BOTH outputs; no
collectives (the contracted-side projection is replicated).

v4 design (fp8 DoubleRow, deep-pipelined):
- All big matmuls (scores, aggregation, denominator, projections) run in
  fp8e4 with perf_mode=DoubleRow (2 k-chunks per instruction, ~1.5x PE).
- Feature matrices are projected twice: fT [h, N] (feature-major, biased,
  used as score lhsT) and vrow [N, h] (row-major, UNbiased, used as
  aggregation rhs). The missing bias in vrow cancels through softmax:
  P@(v + 1 b^T)/rsum = P@vrow/rsum + b^T, so b_feat is folded into the
  residual qrow instead. This removes all per-block PE transposes.
- Per-core inputs are column-ROLLED so this core's rows are columns
  [0:RB) of both feature matrices; the score rhs (qTb) is then just
  fT_other[:, :, 0:RB] - no separate query projection.
- exp uses bias -ln(32): softmax is shift-invariant, masked entries
  become exactly 1/32 (fp8-exact), max value ~5 stays far below fp8e4
  max 240.
- Residual path stays accurate: qrow = f32r projection of the f32 query
  rows -> bf16, + (b_q + b_feat) broadcast row.
- v4 scheduling: fp8 projections run FIRST (first matmul only needs
  ~0.5MB of DMA, not 3.5MB); qrow streams per-column-chunk; PSUM is
  organized as 8 one-bank tiles so projections get a 7-deep ring; the
  attention epilogue + final aggregation of each row-block are deferred
  into the next block's score loop so the PE never drains at block
  boundaries.
"""

import sys

sys.path.insert(0, "/opt/trn_rl_repo")

import ml_dtypes
import numpy as np

import concourse.bacc as bacc
import concourse.bass as bass
import concourse.mybir as mybir
import concourse.tile as tile
from concourse.bass_utils import run_bass_kernel_spmd

N = 8192          # users == items
H = 512           # hidden
NCORES = 8
RB = N // NCORES  # 1024 rows per core per direction
KH = H // 128     # 4 h-chunks
NB = N // 128     # 64 column chunks
NBP = NB // 2     # 32 column-pair chunks (DoubleRow)
NRB = RB // 512   # 2 r-blocks of 512
NJ = N // 512     # 16 512-col blocks for projection streaming
SCALE = float(1.0 / np.sqrt(np.float32(H)))
NLN32 = float(-np.log(32.0))

F32 = mybir.dt.float32
F32R = mybir.dt.float32r
BF16 = mybir.dt.bfloat16
FP8 = mybir.dt.float8e4
NP_FP8 = ml_dtypes.float8_e4m3
DR = mybir.MatmulPerfMode.DoubleRow


def build_nc():
    nc = bacc.Bacc("TRN2", target_bir_lowering=False, debug=False)

    featA = nc.declare_dram_parameter("featA", [H, N], FP8, isOutput=False)
    featB = nc.declare_dram_parameter("featB", [H, N], FP8, isOutput=False)
    qtA = nc.declare_dram_parameter("qtA", [4, 128, KH * 256], F32,
                                    isOutput=False)
    qtB = nc.declare_dram_parameter("qtB", [4, 128, KH * 256], F32,
                                    isOutput=False)
    maskA = nc.declare_dram_parameter("maskA", [NB, 128, RB], FP8, isOutput=False)
    maskB = nc.declare_dram_parameter("maskB", [NB, 128, RB], FP8, isOutput=False)
    WfA = nc.declare_dram_parameter("WfA", [128, KH, H], FP8, isOutput=False)
    WfB = nc.declare_dram_parameter("WfB", [128, KH, H], FP8, isOutput=False)
    WqA = nc.declare_dram_parameter("WqA", [H, H], F32, isOutput=False)
    WqB = nc.declare_dram_parameter("WqB", [H, H], F32, isOutput=False)
    bfA = nc.declare_dram_parameter("bfA", [128, KH], F32, isOutput=False)
    bfB = nc.declare_dram_parameter("bfB", [128, KH], F32, isOutput=False)
    brow = nc.declare_dram_parameter("brow", [128, H], F32, isOutput=False)
    out = nc.declare_dram_parameter("out", [2 * RB, H], F32, isOutput=True)

    with tile.TileContext(nc) as tc:
        with (
            tc.tile_pool(name="bigA", bufs=1) as bigA,
            tc.tile_pool(name="bigB", bufs=1) as bigB,
            tc.tile_pool(name="wts", bufs=1) as wts,
            tc.tile_pool(name="wtsB", bufs=1) as wtsB,
            tc.tile_pool(name="stream", bufs=6) as stream,
            tc.tile_pool(name="qc", bufs=2) as qcp,
            tc.tile_pool(name="mask", bufs=3) as maskp,
            tc.tile_pool(name="pf", bufs=3) as pfp,
            tc.tile_pool(name="pb", bufs=3) as pbp,
            tc.tile_pool(name="outs", bufs=1) as outsp,
            tc.tile_pool(name="small", bufs=1) as small,
            # 8 PSUM banks as single-bank tiles: ps_a holds the 4 agg
            # accumulators during attention (and joins the projection
            # ring before that), ps_b holds score tiles, ps_rs the
            # denominator.
            tc.tile_pool(name="ps_a", bufs=4, space="PSUM") as ps_a,
            tc.tile_pool(name="ps_b", bufs=3, space="PSUM") as ps_b,
            tc.tile_pool(name="ps_rs", bufs=1, space="PSUM") as ps_rs,
        ):
            ones2 = small.tile([128, 2, 16], FP8, tag="ones")
            nc.vector.memset(ones2[:], 1.0)
            nbias = small.tile([128, 1], F32, tag="nbias")
            nc.vector.memset(nbias[:], NLN32)
            brow_sb = small.tile([128, H], F32, tag="brow")
            bfA_sb = small.tile([128, KH], F32, tag="bfA")
            nc.sync.dma_start(bfA_sb[:], bfA[:])
            bfB_sb = small.tile([128, KH], F32, tag="bfB")

            # persistent per-direction tensors
            fT = {}
            vrow = {}
            qrow = {}
            for big_pool, d in ((bigA, "A"), (bigB, "B")):
                fT[d] = big_pool.tile([128, KH, N], FP8, tag=f"fT{d}",
                                      name=f"fT{d}")
                vrow[d] = big_pool.tile([128, NB, H], FP8, tag=f"vrow{d}",
                                        name=f"vrow{d}")
                qrow[d] = big_pool.tile([128, 2 * KH, H], BF16, tag=f"qrow{d}",
                                        name=f"qrow{d}")

            # 7-deep psum ring for the projection phase (attention pins
            # ps_a's 4 bufs as accumulators, projections may rotate
            # through everything).
            _ring = [0]

            def proj_ps():
                _ring[0] += 1
                pool = (ps_a, ps_b)[_ring[0] % 2]
                ps = pool.tile([128, 512], F32, tag="s",
                               name=f"ps{_ring[0]}")
                return ps

            # ---------------- phase 0a: fp8 projections ----------------
            def project_fp8(d, feat_dram, wf_dram, bias_f, side_tasks=()):
                side_tasks = list(side_tasks)
                wfp = wts.tile([128, KH, H], FP8, tag=f"wfp{d}",
                               name=f"wfp{d}")
                nc.sync.dma_start(wfp[:], wf_dram[:])
                for j in range(NJ):
                    ft_in = stream.tile([128, KH, 512], FP8, tag="ft",
                                        name=f"ft{d}{j}")
                    for k in range(KH):
                        nc.sync.dma_start(
                            ft_in[:, k, :],
                            feat_dram[k * 128:(k + 1) * 128,
                                      j * 512:(j + 1) * 512])
                    for m in range(KH):
                        ps = proj_ps()
                        for ko in range(2):
                            nc.tensor.matmul(
                                ps[:],
                                wfp[:, 2 * ko:2 * ko + 2, m * 128:(m + 1) * 128],
                                ft_in[:, 2 * ko:2 * ko + 2, :],
                                start=(ko == 0), stop=(ko == 1), perf_mode=DR)
                        if m % 2 == 0:
                            nc.vector.tensor_scalar(
                                out=fT[d][:, m, j * 512:(j + 1) * 512],
                                in0=ps[:], scalar1=bias_f[:, m:m + 1],
                                scalar2=None, op0=mybir.AluOpType.add)
                        else:
                            nc.scalar.add(
                                fT[d][:, m, j * 512:(j + 1) * 512], ps[:],
                                bias_f[:, m:m + 1])
                    for sub in range(4):
                        c = j * 4 + sub
                        ps = proj_ps()
                        for ko in range(2):
                            nc.tensor.matmul(
                                ps[:],
                                ft_in[:, 2 * ko:2 * ko + 2,
                                      sub * 128:(sub + 1) * 128],
                                wfp[:, 2 * ko:2 * ko + 2, :],
                                start=(ko == 0), stop=(ko == 1), perf_mode=DR)
                        if sub % 2 == 0:
                            nc.vector.tensor_copy(vrow[d][:, c, :], ps[:])
                        else:
                            nc.scalar.copy(vrow[d][:, c, :], ps[:])
                    # one interleaved residual-projection task per j-block:
                    # fills the DMA-gated moments of this stream with the
                    # qrow f32r matmuls, eliminating a separate qrow phase.
                    if side_tasks:
                        side_tasks.pop(0)()

            # ---------------- phase 0b: residual projections ----------------
            # wq for A lives in the wts pool; wq for B reuses the (now
            # idle) fp8 feature-stream pool slots, so its DMA can land
            # while qrow A computes.
            def load_wq(d, wq_dram, pool):
                wq = [pool.tile([128, H], F32R, tag=f"wq{k}", name=f"wq{d}{k}")
                      for k in range(KH)]
                for k in range(KH):
                    nc.sync.dma_start(
                        wq[k][:], wq_dram[k * 128:(k + 1) * 128, :].bitcast(F32R))
                return wq

            def qrow_task(d, qt_dram, c, state):
                def task():
                    c2, cc = divmod(c, 2)
                    if cc == 0:
                        qc = qcp.tile([128, KH, 256], F32R, tag="qt",
                                      name=f"qt{d}{c2}")
                        nc.sync.dma_start(
                            qc[:], qt_dram[c2].bitcast(F32R).rearrange(
                                "p (k c) -> p k c", k=KH))
                        state["qc"] = qc
                    qc = state["qc"]
                    wq = wq_sb[d]
                    ps = proj_ps()
                    for k in range(KH):
                        nc.tensor.matmul(
                            ps[:], qc[:, k, cc * 128:(cc + 1) * 128],
                            wq[k][:],
                            start=(k == 0), stop=(k == KH - 1))
                    nc.vector.tensor_tensor(
                        out=qrow[d][:, c, :], in0=ps[:], in1=brow_sb[:],
                        op=mybir.AluOpType.add)
                return task

            project_fp8("A", featA, WfA, bfA_sb)
            # wq DMAs hoisted here: they execute behind projection A's
            # feature stream, well before the qrow tasks need them.
            wq_sb = {"A": load_wq("A", WqA, wts), "B": load_wq("B", WqB, wtsB)}
            # brow (first used by the qrow adds, t~75us) and bfB (first
            # used by projection B, t~73us) dispatch here, off the startup
            # critical path: they execute behind projection A's stream.
            nc.sync.dma_start(brow_sb[:], brow[:])
            nc.sync.dma_start(bfB_sb[:], bfB[:])
            _qstate = {}
            project_fp8("B", featB, WfB, bfB_sb, side_tasks=(
                [qrow_task("A", qtA, c, _qstate) for c in range(2 * KH)]
                + [qrow_task("B", qtB, c, _qstate) for c in range(2 * KH)]))

            # ---------------- attention ----------------
            # Flat schedule over 4 row-blocks (A rb0, A rb1, B rb0, B rb1).
            # The final aggregation (pend) and the epilogue (prev) of each
            # block are emitted inside the NEXT block's score loop, so the
            # PE keeps streaming across boundaries.

            def emit_agg(agg4, rsum4, pbf2, bp, myvrow, rsum_first=False):
                if rsum_first:
                    # final flush only: the epilogue's recip waits on the
                    # rsum stop, so complete the denominator before the
                    # agg matmuls (steady-state order stays interleaved -
                    # reordering it regresses PE pacing).
                    for rs in range(4):
                        nc.tensor.matmul(
                            rsum4[:, rs:rs + 1],
                            pbf2[:, :, rs * 128:(rs + 1) * 128],
                            ones2[:, :, 0:1],
                            start=(bp == 0 and rs == 0),
                            stop=(bp == NBP - 1 and rs == 3), perf_mode=DR)
                    for rs in range(4):
                        nc.tensor.matmul(
                            agg4[rs][:], pbf2[:, :, rs * 128:(rs + 1) * 128],
                            myvrow[:, 2 * bp:2 * bp + 2, :],
                            start=(bp == 0), stop=(bp == NBP - 1),
                            perf_mode=DR)
                    return
                for rs in range(4):
                    nc.tensor.matmul(
                        agg4[rs][:], pbf2[:, :, rs * 128:(rs + 1) * 128],
                        myvrow[:, 2 * bp:2 * bp + 2, :],
                        start=(bp == 0), stop=(bp == NBP - 1), perf_mode=DR)
                    # all 4 columns form ONE psum accumulation group (they
                    # share a 2KB zero region): start only on the very first
                    # matmul, stop only on the very last
                    nc.tensor.matmul(
                        rsum4[:, rs:rs + 1],
                        pbf2[:, :, rs * 128:(rs + 1) * 128],
                        ones2[:, :, 0:1],
                        start=(bp == 0 and rs == 0),
                        stop=(bp == NBP - 1 and rs == 3), perf_mode=DR)

            def emit_epilogue(d, rb, agg4, rsum4, out_base, final=False):
                recip = small.tile([128, 4], F32, tag="recip")
                nc.vector.reciprocal(recip[:], rsum4[:])
                for rs in range(4):
                    o_sb = outsp.tile([128, H], F32, tag=f"o{rs}",
                                      name=f"o{d}{rb}_{rs}")
                    if rs % 2 == 0:
                        nc.vector.tensor_scalar(
                            out=o_sb[:], in0=agg4[rs][:],
                            scalar1=recip[:, rs:rs + 1], scalar2=None,
                            op0=mybir.AluOpType.mult)
                    else:
                        nc.scalar.mul(o_sb[:], agg4[rs][:],
                                      recip[:, rs:rs + 1])
                    # the +qrow adds touch only SBUF, so they go on the
                    # idle GpSimd (mid-kernel: keeps DVE/Scalar clear for
                    # the PSUM reads the next block's agg WAR-waits on);
                    # the FINAL epilogue is a serial tail, so split its
                    # adds with DVE (GpSimd adds are ~1.6us each).
                    add_eng = nc.vector if (final and rs % 2 == 0) \
                        else nc.gpsimd
                    add_eng.tensor_tensor(
                        out=o_sb[:], in0=o_sb[:],
                        in1=qrow[d][:, rb * 4 + rs, :],
                        op=mybir.AluOpType.add)
                    row0 = out_base + rb * 512 + rs * 128
                    # final epilogue: spread the four out-DMAs across four
                    # engine queues so the 256KB transfers run in parallel
                    # instead of serializing behind one dispatch queue -
                    # the end-of-kernel barrier waits on the last packet.
                    dma_eng = (nc.sync, nc.scalar, nc.gpsimd,
                               nc.scalar)[rs] if final else nc.sync
                    dma_eng.dma_start(out[row0:row0 + 128, :], o_sb[:])

            blocks = [(d, other, mask_dram, out_base, rb)
                      for (d, other, mask_dram, out_base) in
                      (("A", "B", maskA, 0), ("B", "A", maskB, RB))
                      for rb in range(NRB)]

            pendq = []   # (agg4, rsum4, pbf2, bp, vrow) awaiting aggregation
            prev = None  # (d, rb, agg4, rsum4, out_base) awaiting epilogue
            for bi, (d, other, mask_dram, out_base, rb) in enumerate(blocks):
                myfT = fT[d]
                qTb = fT[other]
                boundary = bi > 0
                agg4 = [ps_a.tile([128, 512], F32, tag="s",
                                  name=f"agg{d}{rb}_{rs}") for rs in range(4)]
                rsum4 = ps_rs.tile([128, 4], F32, tag="rs")
                for bp in range(NBP):
                    sps = []
                    for t in range(2):
                        b = 2 * bp + t
                        sp = ps_b.tile([128, 512], F32, tag="s")
                        for ko in range(2):
                            nc.tensor.matmul(
                                sp[:],
                                myfT[:, 2 * ko:2 * ko + 2,
                                     b * 128:(b + 1) * 128],
                                qTb[:, 2 * ko:2 * ko + 2,
                                    rb * 512:(rb + 1) * 512],
                                start=(ko == 0), stop=(ko == 1),
                                perf_mode=DR)
                        sps.append(sp)

                    # aggregate earlier pairs while DVE/Act chew on this
                    # one. At a block boundary: flush the old block's last
                    # aggregation + epilogue at bp0, then hold this
                    # block's first aggregations until bp2 so the new
                    # PSUM accumulation group doesn't wait on the old
                    # epilogue's reads.
                    if pendq:
                        emit_agg(*pendq.pop(0))
                    if prev is not None:
                        emit_epilogue(*prev)
                        prev = None
                    mt = maskp.tile([128, 2, 512], FP8, tag="mk")
                    nc.sync.dma_start(
                        mt[:],
                        mask_dram[2 * bp:2 * bp + 2, :,
                                  rb * 512:(rb + 1) * 512].rearrange(
                                      "t p c -> p t c"))
                    pbf2 = pbp.tile([128, 2, 512], FP8, tag="pbf")
                    for t in range(2):
                        # gpsimd cannot read PSUM; both mults go on DVE
                        p32 = pfp.tile([128, 512], F32, tag="p32")
                        nc.vector.tensor_tensor(
                            out=p32[:], in0=sps[t][:], in1=mt[:, t, :],
                            op=mybir.AluOpType.mult)
                        nc.scalar.activation(
                            pbf2[:, t, :], p32[:],
                            mybir.ActivationFunctionType.Exp,
                            bias=nbias[:], scale=SCALE)
                    pendq.append((agg4, rsum4, pbf2, bp, vrow[d]))
                prev = (d, rb, agg4, rsum4, out_base)
            while pendq:
                emit_agg(*pendq.pop(0), rsum_first=True)
            emit_epilogue(*prev, final=True)

    nc.compile()
    return nc


_NC_CACHE = None
TRACE = False
LAST_RESULT = None


def kernel(user, item, UV_adj, VU_adj, W_u, b_u, W_v, b_v):
    global _NC_CACHE, LAST_RESULT
    user = np.asarray(user, dtype=np.float32)
    item = np.asarray(item, dtype=np.float32)
    UV_adj = np.asarray(UV_adj, dtype=np.float32)
    VU_adj = np.asarray(VU_adj, dtype=np.float32)
    W_u = np.asarray(W_u, dtype=np.float32)
    W_v = np.asarray(W_v, dtype=np.float32)
    b_u = np.asarray(b_u, dtype=np.float32)
    b_v = np.asarray(b_v, dtype=np.float32)

    userT = np.ascontiguousarray(user.T)
    itemT = np.ascontiguousarray(item.T)
    userT8 = userT.astype(NP_FP8)
    itemT8 = itemT.astype(NP_FP8)
    UV8 = UV_adj.astype(NP_FP8)
    VU8 = np.ascontiguousarray(UV8.T)
    W_uT = np.ascontiguousarray(W_u.T)
    W_vT = np.ascontiguousarray(W_v.T)
    # [128, KH, H] fp8 weight layout for DoubleRow projections
    WfA_np = np.ascontiguousarray(
        W_vT.reshape(KH, 128, H).transpose(1, 0, 2).astype(NP_FP8))
    WfB_np = np.ascontiguousarray(
        W_uT.reshape(KH, 128, H).transpose(1, 0, 2).astype(NP_FP8))
    bfA_np = np.ascontiguousarray(b_v.reshape(KH, 128).T)
    bfB_np = np.ascontiguousarray(b_u.reshape(KH, 128).T)
    brow_np = np.ascontiguousarray(
        np.broadcast_to((b_u + b_v)[None, :], (128, H)))

    in_maps = []
    for i in range(NCORES):
        r = i * RB
        sl = slice(r, r + RB)
        in_maps.append({
            # feature matrices with this core's rows rolled to the front
            "featA": np.ascontiguousarray(np.roll(itemT8, -r, axis=1)),
            "featB": np.ascontiguousarray(np.roll(userT8, -r, axis=1)),
            "qtA": np.ascontiguousarray(
                userT[:, sl].reshape(KH, 128, 4, 256).transpose(2, 1, 0, 3)
                .reshape(4, 128, KH * 256)),
            "qtB": np.ascontiguousarray(
                itemT[:, sl].reshape(KH, 128, 4, 256).transpose(2, 1, 0, 3)
                .reshape(4, 128, KH * 256)),
            "maskA": np.ascontiguousarray(
                np.roll(VU8[:, sl], -r, axis=0)).reshape(NB, 128, RB),
            "maskB": np.ascontiguousarray(
                np.roll(UV8[:, sl], -r, axis=0)).reshape(NB, 128, RB),
            "WfA": WfA_np,
            "WfB": WfB_np,
            "WqA": W_uT,
            "WqB": W_vT,
            "bfA": bfA_np,
            "bfB": bfB_np,
            "brow": brow_np,
        })

    if _NC_CACHE is None:
        _NC_CACHE = build_nc()
    res = run_bass_kernel_spmd(_NC_CACHE, in_maps, core_ids=list(range(NCORES)),
                               trace=TRACE)
    LAST_RESULT = res
    results = res.results
    learn_user = np.concatenate([results[i]["out"][:RB] for i in range(NCORES)], 0)
    learn_item = np.concatenate([results[i]["out"][RB:] for i in range(NCORES)], 0)
    return (learn_user, learn_item)


if __name__ == "__main__":
    nc = build_nc()
    print("built ok")


# revision 35
# speedup vs baseline: 1.0419x; 1.0054x over previous
"""Bipartite GNN attention kernel for Trainium2, SPMD across 8 NeuronCores.

Math (per reference):
  u = user @ W_u.T + b_u ; v = item @ W_v.T + b_v
  learn_user = softmax((u @ v.T) * UV_adj * scale, axis=1) @ v + u
  learn_item = softmax((v @ u.T) * VU_adj * scale, axis=1) @ u + v

Sharding: core i owns rows [i*1024, (i+1)*1024) of BOTH outputs; no
collectives (the contracted-side projection is replicated).

v4 design (fp8 DoubleRow, deep-pipelined):
- All big matmuls (scores, aggregation, denominator, projections) run in
  fp8e4 with perf_mode=DoubleRow (2 k-chunks per instruction, ~1.5x PE).
- Feature matrices are projected twice: fT [h, N] (feature-major, biased,
  used as score lhsT) and vrow [N, h] (row-major, UNbiased, used as
  aggregation rhs). The missing bias in vrow cancels through softmax:
  P@(v + 1 b^T)/rsum = P@vrow/rsum + b^T, so b_feat is folded into the
  residual qrow instead. This removes all per-block PE transposes.
- Per-core inputs are column-ROLLED so this core's rows are columns
  [0:RB) of both feature matrices; the score rhs (qTb) is then just
  fT_other[:, :, 0:RB] - no separate query projection.
- exp uses bias -ln(32): softmax is shift-invariant, masked entries
  become exactly 1/32 (fp8-exact), max value ~5 stays far below fp8e4
  max 240.
- Residual path stays accurate: qrow = f32r projection of the f32 query
  rows -> bf16, + (b_q + b_feat) broadcast row.
- v4 scheduling: fp8 projections run FIRST (first matmul only needs
  ~0.5MB of DMA, not 3.5MB); qrow streams per-column-chunk; PSUM is
  organized as 8 one-bank tiles so projections get a 7-deep ring; the
  attention epilogue + final aggregation of each row-block are deferred
  into the next block's score loop so the PE never drains at block
  boundaries.
"""

import sys

sys.path.insert(0, "/opt/trn_rl_repo")

import ml_dtypes
import numpy as np

import concourse.bacc as bacc
import concourse.bass as bass
import concourse.mybir as mybir
import concourse.tile as tile
from concourse.bass_utils import run_bass_kernel_spmd

N = 8192          # users == items
H = 512           # hidden
NCORES = 8
RB = N // NCORES  # 1024 rows per core per direction
KH = H // 128     # 4 h-chunks
NB = N // 128     # 64 column chunks
NBP = NB // 2     # 32 column-pair chunks (DoubleRow)
NRB = RB // 512   # 2 r-blocks of 512
NJ = N // 512     # 16 512-col blocks for projection streaming
SCALE = float(1.0 / np.sqrt(np.float32(H)))
NLN32 = float(-np.log(32.0))

F32 = mybir.dt.float32
F32R = mybir.dt.float32r
BF16 = mybir.dt.bfloat16
FP8 = mybir.dt.float8e4
NP_FP8 = ml_dtypes.float8_e4m3
DR = mybir.MatmulPerfMode.DoubleRow


def build_nc():
    nc = bacc.Bacc("TRN2", target_bir_lowering=False, debug=False)

    featA = nc.declare_dram_parameter("featA", [H, N], FP8, isOutput=False)
    featB = nc.declare_dram_parameter("featB", [H, N], FP8, isOutput=False)
    qtA = nc.declare_dram_parameter("qtA", [4, 128, KH * 256], F32,
                                    isOutput=False)
    qtB = nc.declare_dram_parameter("qtB", [4, 128, KH * 256], F32,
                                    isOutput=False)
    maskA = nc.declare_dram_parameter("maskA", [NB, 128, RB], FP8, isOutput=False)
    maskB = nc.declare_dram_parameter("maskB", [NB, 128, RB], FP8, isOutput=False)
    WfA = nc.declare_dram_parameter("WfA", [128, KH, H], FP8, isOutput=False)
    WfB = nc.declare_dram_parameter("WfB", [128, KH, H], FP8, isOutput=False)
    WqA = nc.declare_dram_parameter("WqA", [H, H], F32, isOutput=False)
    WqB = nc.declare_dram_parameter("WqB", [H, H], F32, isOutput=False)
    bfA = nc.declare_dram_parameter("bfA", [128, KH], F32, isOutput=False)
    bfB = nc.declare_dram_parameter("bfB", [128, KH], F32, isOutput=False)
    brow = nc.declare_dram_parameter("brow", [128, H], F32, isOutput=False)
    out = nc.declare_dram_parameter("out", [2 * RB, H], F32, isOutput=True)

    with tile.TileContext(nc) as tc:
        with (
            tc.tile_pool(name="bigA", bufs=1) as bigA,
            tc.tile_pool(name="bigB", bufs=1) as bigB,
            tc.tile_pool(name="wts", bufs=1) as wts,
            tc.tile_pool(name="wtsB", bufs=1) as wtsB,
            tc.tile_pool(name="stream", bufs=6) as stream,
            tc.tile_pool(name="qc", bufs=2) as qcp,
            tc.tile_pool(name="mask", bufs=3) as maskp,
            tc.tile_pool(name="pf", bufs=3) as pfp,
            tc.tile_pool(name="pb", bufs=3) as pbp,
            tc.tile_pool(name="outs", bufs=1) as outsp,
            tc.tile_pool(name="small", bufs=1) as small,
            # 8 PSUM banks as single-bank tiles: ps_a holds the 4 agg
            # accumulators during attention (and joins the projection
            # ring before that), ps_b holds score tiles, ps_rs the
            # denominator.
            tc.tile_pool(name="ps_a", bufs=4, space="PSUM") as ps_a,
            tc.tile_pool(name="ps_b", bufs=3, space="PSUM") as ps_b,
            tc.tile_pool(name="ps_rs", bufs=1, space="PSUM") as ps_rs,
        ):
            ones2 = small.tile([128, 2, 16], FP8, tag="ones")
            nc.vector.memset(ones2[:], 1.0)
            nbias = small.tile([128, 1], F32, tag="nbias")
            nc.vector.memset(nbias[:], NLN32)
            brow_sb = small.tile([128, H], F32, tag="brow")
            bfA_sb = small.tile([128, KH], F32, tag="bfA")
            nc.sync.dma_start(bfA_sb[:], bfA[:])
            bfB_sb = small.tile([128, KH], F32, tag="bfB")

            # persistent per-direction tensors
            fT = {}
            vrow = {}
            qrow = {}
            for big_pool, d in ((bigA, "A"), (bigB, "B")):
                fT[d] = big_pool.tile([128, KH, N], FP8, tag=f"fT{d}",
                                      name=f"fT{d}")
                vrow[d] = big_pool.tile([128, NB, H], FP8, tag=f"vrow{d}",
                                        name=f"vrow{d}")
                qrow[d] = big_pool.tile([128, 2 * KH, H], BF16, tag=f"qrow{d}",
                                        name=f"qrow{d}")

            # 7-deep psum ring for the projection phase (attention pins
            # ps_a's 4 bufs as accumulators, projections may rotate
            # through everything).
            _ring = [0]

            def proj_ps():
                _ring[0] += 1
                pool = (ps_a, ps_b)[_ring[0] % 2]
                ps = pool.tile([128, 512], F32, tag="s",
                               name=f"ps{_ring[0]}")
                return ps

            # ---------------- phase 0a: fp8 projections ----------------
            def project_fp8(d, feat_dram, wf_dram, bias_f, side_tasks=()):
                side_tasks = list(side_tasks)
                wfp = wts.tile([128, KH, H], FP8, tag=f"wfp{d}",
                               name=f"wfp{d}")
                nc.sync.dma_start(wfp[:], wf_dram[:])
                for j in range(NJ):
                    ft_in = stream.tile([128, KH, 512], FP8, tag="ft",
                                        name=f"ft{d}{j}")
                    for k in range(KH):
                        nc.sync.dma_start(
                            ft_in[:, k, :],
                            feat_dram[k * 128:(k + 1) * 128,
                                      j * 512:(j + 1) * 512])
                    for m in range(KH):
                        ps = proj_ps()
                        for ko in range(2):
                            nc.tensor.matmul(
                                ps[:],
                                wfp[:, 2 * ko:2 * ko + 2, m * 128:(m + 1) * 128],
                                ft_in[:, 2 * ko:2 * ko + 2, :],
                                start=(ko == 0), stop=(ko == 1), perf_mode=DR)
                        if m % 2 == 0:
                            nc.vector.tensor_scalar(
                                out=fT[d][:, m, j * 512:(j + 1) * 512],
                                in0=ps[:], scalar1=bias_f[:, m:m + 1],
                                scalar2=None, op0=mybir.AluOpType.add)
                        else:
                            nc.scalar.add(
                                fT[d][:, m, j * 512:(j + 1) * 512], ps[:],
                                bias_f[:, m:m + 1])
                    for sub in range(4):
                        c = j * 4 + sub
                        ps = proj_ps()
                        for ko in range(2):
                            nc.tensor.matmul(
                                ps[:],
                                ft_in[:, 2 * ko:2 * ko + 2,
                                      sub * 128:(sub + 1) * 128],
                                wfp[:, 2 * ko:2 * ko + 2, :],
                                start=(ko == 0), stop=(ko == 1), perf_mode=DR)
                        if sub % 2 == 0:
                            nc.vector.tensor_copy(vrow[d][:, c, :], ps[:])
                        else:
                            nc.scalar.copy(vrow[d][:, c, :], ps[:])
                    # one interleaved residual-projection task per j-block:
                    # fills the DMA-gated moments of this stream with the
                    # qrow f32r matmuls, eliminating a separate qrow phase.
                    if side_tasks:
                        side_tasks.pop(0)()

            # ---------------- phase 0b: residual projections ----------------
            # wq for A lives in the wts pool; wq for B reuses the (now
            # idle) fp8 feature-stream pool slots, so its DMA can land
            # while qrow A computes.
            def load_wq(d, wq_dram, pool):
                wq = [pool.tile([128, H], F32R, tag=f"wq{k}", name=f"wq{d}{k}")
                      for k in range(KH)]
                for k in range(KH):
                    nc.sync.dma_start(
                        wq[k][:], wq_dram[k * 128:(k + 1) * 128, :].bitcast(F32R))
                return wq

            def qrow_task(d, qt_dram, c, state):
                def task():
                    c2, cc = divmod(c, 2)
                    if cc == 0:
                        qc = qcp.tile([128, KH, 256], F32R, tag="qt",
                                      name=f"qt{d}{c2}")
                        nc.sync.dma_start(
                            qc[:], qt_dram[c2].bitcast(F32R).rearrange(
                                "p (k c) -> p k c", k=KH))
                        state["qc"] = qc
                    qc = state["qc"]
                    wq = wq_sb[d]
                    ps = proj_ps()
                    for k in range(KH):
                        nc.tensor.matmul(
                            ps[:], qc[:, k, cc * 128:(cc + 1) * 128],
                            wq[k][:],
                            start=(k == 0), stop=(k == KH - 1))
                    nc.vector.tensor_tensor(
                        out=qrow[d][:, c, :], in0=ps[:], in1=brow_sb[:],
                        op=mybir.AluOpType.add)
                return task

            project_fp8("A", featA, WfA, bfA_sb)
            # wq DMAs hoisted here: they execute behind projection A's
            # feature stream, well before the qrow tasks need them.
            wq_sb = {"A": load_wq("A", WqA, wts), "B": load_wq("B", WqB, wtsB)}
            # brow (first used by the qrow adds, t~75us) and bfB (first
            # used by projection B, t~73us) dispatch here, off the startup
            # critical path: they execute behind projection A's stream.
            nc.sync.dma_start(brow_sb[:], brow[:])
            nc.sync.dma_start(bfB_sb[:], bfB[:])
            _qstate = {}
            project_fp8("B", featB, WfB, bfB_sb, side_tasks=(
                [qrow_task("A", qtA, c, _qstate) for c in range(2 * KH)]
                + [qrow_task("B", qtB, c, _qstate) for c in range(2 * KH)]))

            # ---------------- attention ----------------
            # Flat schedule over 4 row-blocks (A rb0, A rb1, B rb0, B rb1).
            # The final aggregation (pend) and the epilogue (prev) of each
            # block are emitted inside the NEXT block's score loop, so the
            # PE keeps streaming across boundaries.

            def emit_agg(agg4, rsum4, pbf2, bp, myvrow, rsum_first=False):
                if rsum_first:
                    # final flush only: the epilogue's recip waits on the
                    # rsum stop, so complete the denominator before the
                    # agg matmuls (steady-state order stays interleaved -
                    # reordering it regresses PE pacing).
                    for rs in range(4):
                        nc.tensor.matmul(
                            rsum4[:, rs:rs + 1],
                            pbf2[:, :, rs * 128:(rs + 1) * 128],
                            ones2[:, :, 0:1],
                            start=(bp == 0 and rs == 0),
                            stop=(bp == NBP - 1 and rs == 3), perf_mode=DR)
                    for rs in range(4):
                        nc.tensor.matmul(
                            agg4[rs][:], pbf2[:, :, rs * 128:(rs + 1) * 128],
                            myvrow[:, 2 * bp:2 * bp + 2, :],
                            start=(bp == 0), stop=(bp == NBP - 1),
                            perf_mode=DR)
                    return
                for rs in range(4):
                    nc.tensor.matmul(
                        agg4[rs][:], pbf2[:, :, rs * 128:(rs + 1) * 128],
                        myvrow[:, 2 * bp:2 * bp + 2, :],
                        start=(bp == 0), stop=(bp == NBP - 1), perf_mode=DR)
                    # all 4 columns form ONE psum accumulation group (they
                    # share a 2KB zero region): start only on the very first
                    # matmul, stop only on the very last
                    nc.tensor.matmul(
                        rsum4[:, rs:rs + 1],
                        pbf2[:, :, rs * 128:(rs + 1) * 128],
                        ones2[:, :, 0:1],
                        start=(bp == 0 and rs == 0),
                        stop=(bp == NBP - 1 and rs == 3), perf_mode=DR)

            def emit_epilogue(d, rb, agg4, rsum4, out_base, final=False):
                recip = small.tile([128, 4], F32, tag="recip")
                nc.vector.reciprocal(recip[:], rsum4[:])
                for rs in range(4):
                    o_sb = outsp.tile([128, H], F32, tag=f"o{rs}",
                                      name=f"o{d}{rb}_{rs}")
                    if rs % 2 == 0:
                        nc.vector.tensor_scalar(
                            out=o_sb[:], in0=agg4[rs][:],
                            scalar1=recip[:, rs:rs + 1], scalar2=None,
                            op0=mybir.AluOpType.mult)
                    else:
                        nc.scalar.mul(o_sb[:], agg4[rs][:],
                                      recip[:, rs:rs + 1])
                    # the +qrow adds touch only SBUF, so they go on the
                    # idle GpSimd (mid-kernel: keeps DVE/Scalar clear for
                    # the PSUM reads the next block's agg WAR-waits on);
                    # the FINAL epilogue is a serial tail, so split its
                    # adds with DVE (GpSimd adds are ~1.6us each).
                    add_eng = nc.vector if (final and rs % 2 == 0) \
                        else nc.gpsimd
                    add_eng.tensor_tensor(
                        out=o_sb[:], in0=o_sb[:],
                        in1=qrow[d][:, rb * 4 + rs, :],
                        op=mybir.AluOpType.add)
                    row0 = out_base + rb * 512 + rs * 128
                    # final epilogue: spread the four out-DMAs across four
                    # engine queues so the 256KB transfers run in parallel
                    # instead of serializing behind one dispatch queue -
                    # the end-of-kernel barrier waits on the last packet.
                    # mid-kernel: out-DMAs ride the GpSimd queue (it just
                    # did the adds and then idles) so the sync queue stays
                    # clear for the next block's mask stream at boundaries
                    dma_eng = (nc.sync, nc.scalar, nc.gpsimd,
                               nc.scalar)[rs] if final else nc.gpsimd
                    dma_eng.dma_start(out[row0:row0 + 128, :], o_sb[:])

            blocks = [(d, other, mask_dram, out_base, rb)
                      for (d, other, mask_dram, out_base) in
                      (("A", "B", maskA, 0), ("B", "A", maskB, RB))
                      for rb in range(NRB)]

            pendq = []   # (agg4, rsum4, pbf2, bp, vrow) awaiting aggregation
            prev = None  # (d, rb, agg4, rsum4, out_base) awaiting epilogue
            for bi, (d, other, mask_dram, out_base, rb) in enumerate(blocks):
                myfT = fT[d]
                qTb = fT[other]
                boundary = bi > 0
                agg4 = [ps_a.tile([128, 512], F32, tag="s",
                                  name=f"agg{d}{rb}_{rs}") for rs in range(4)]
                rsum4 = ps_rs.tile([128, 4], F32, tag="rs")
                for bp in range(NBP):
                    sps = []
                    for t in range(2):
                        b = 2 * bp + t
                        sp = ps_b.tile([128, 512], F32, tag="s")
                        for ko in range(2):
                            nc.tensor.matmul(
                                sp[:],
                                myfT[:, 2 * ko:2 * ko + 2,
                                     b * 128:(b + 1) * 128],
                                qTb[:, 2 * ko:2 * ko + 2,
                                    rb * 512:(rb + 1) * 512],
                                start=(ko == 0), stop=(ko == 1),
                                perf_mode=DR)
                        sps.append(sp)

                    # aggregate earlier pairs while DVE/Act chew on this
                    # one. At a block boundary: flush the old block's last
                    # aggregation + epilogue at bp0, then hold this
                    # block's first aggregations until bp2 so the new
                    # PSUM accumulation group doesn't wait on the old
                    # epilogue's reads.
                    if pendq:
                        emit_agg(*pendq.pop(0))
                    if prev is not None:
                        emit_epilogue(*prev)
                        prev = None
                    mt = maskp.tile([128, 2, 512], FP8, tag="mk")
                    nc.sync.dma_start(
                        mt[:],
                        mask_dram[2 * bp:2 * bp + 2, :,
                                  rb * 512:(rb + 1) * 512].rearrange(
                                      "t p c -> p t c"))
                    pbf2 = pbp.tile([128, 2, 512], FP8, tag="pbf")
                    for t in range(2):
                        # gpsimd cannot read PSUM; both mults go on DVE
                        p32 = pfp.tile([128, 512], F32, tag="p32")
                        nc.vector.tensor_tensor(
                            out=p32[:], in0=sps[t][:], in1=mt[:, t, :],
                            op=mybir.AluOpType.mult)
                        nc.scalar.activation(
                            pbf2[:, t, :], p32[:],
                            mybir.ActivationFunctionType.Exp,
                            bias=nbias[:], scale=SCALE)
                    pendq.append((agg4, rsum4, pbf2, bp, vrow[d]))
                prev = (d, rb, agg4, rsum4, out_base)
            while pendq:
                emit_agg(*pendq.pop(0), rsum_first=True)
            emit_epilogue(*prev, final=True)

    nc.compile()
    return nc


_NC_CACHE = None
TRACE = False
LAST_RESULT = None


def kernel(user, item, UV_adj, VU_adj, W_u, b_u, W_v, b_v):
    global _NC_CACHE, LAST_RESULT
    user = np.asarray(user, dtype=np.float32)
    item = np.asarray(item, dtype=np.float32)
    UV_adj = np.asarray(UV_adj, dtype=np.float32)
    VU_adj = np.asarray(VU_adj, dtype=np.float32)
    W_u = np.asarray(W_u, dtype=np.float32)
    W_v = np.asarray(W_v, dtype=np.float32)
    b_u = np.asarray(b_u, dtype=np.float32)
    b_v = np.asarray(b_v, dtype=np.float32)

    userT = np.ascontiguousarray(user.T)
    itemT = np.ascontiguousarray(item.T)
    userT8 = userT.astype(NP_FP8)
    itemT8 = itemT.astype(NP_FP8)
    UV8 = UV_adj.astype(NP_FP8)
    VU8 = np.ascontiguousarray(UV8.T)
    W_uT = np.ascontiguousarray(W_u.T)
    W_vT = np.ascontiguousarray(W_v.T)
    # [128, KH, H] fp8 weight layout for DoubleRow projections
    WfA_np = np.ascontiguousarray(
        W_vT.reshape(KH, 128, H).transpose(1, 0, 2).astype(NP_FP8))
    WfB_np = np.ascontiguousarray(
        W_uT.reshape(KH, 128, H).transpose(1, 0, 2).astype(NP_FP8))
    bfA_np = np.ascontiguousarray(b_v.reshape(KH, 128).T)
    bfB_np = np.ascontiguousarray(b_u.reshape(KH, 128).T)
    brow_np = np.ascontiguousarray(
        np.broadcast_to((b_u + b_v)[None, :], (128, H)))

    in_maps = []
    for i in range(NCORES):
        r = i * RB
        sl = slice(r, r + RB)
        in_maps.append({
            # feature matrices with this core's rows rolled to the front
            "featA": np.ascontiguousarray(np.roll(itemT8, -r, axis=1)),
            "featB": np.ascontiguousarray(np.roll(userT8, -r, axis=1)),
            "qtA": np.ascontiguousarray(
                userT[:, sl].reshape(KH, 128, 4, 256).transpose(2, 1, 0, 3)
                .reshape(4, 128, KH * 256)),
            "qtB": np.ascontiguousarray(
                itemT[:, sl].reshape(KH, 128, 4, 256).transpose(2, 1, 0, 3)
                .reshape(4, 128, KH * 256)),
            "maskA": np.ascontiguousarray(
                np.roll(VU8[:, sl], -r, axis=0)).reshape(NB, 128, RB),
            "maskB": np.ascontiguousarray(
                np.roll(UV8[:, sl], -r, axis=0)).reshape(NB, 128, RB),
            "WfA": WfA_np,
            "WfB": WfB_np,
            "WqA": W_uT,
            "WqB": W_vT,
            "bfA": bfA_np,
            "bfB": bfB_np,
            "brow": brow_np,
        })

    if _NC_CACHE is None:
        _NC_CACHE = build_nc()
    res = run_bass_kernel_spmd(_NC_CACHE, in_maps, core_ids=list(range(NCORES)),
                               trace=TRACE)
    LAST_RESULT = res
    results = res.results
    learn_user = np.concatenate([results[i]["out"][:RB] for i in range(NCORES)], 0)
    learn_item = np.concatenate([results[i]["out"][RB:] for i in range(NCORES)], 0)
    return (learn_user, learn_item)


if __name__ == "__main__":
    nc = build_nc()
    print("built ok")


# revision 36
# speedup vs baseline: 1.0427x; 1.0008x over previous
"""Bipartite GNN attention kernel for Trainium2, SPMD across 8 NeuronCores.

Math (per reference):
  u = user @ W_u.T + b_u ; v = item @ W_v.T + b_v
  learn_user = softmax((u @ v.T) * UV_adj * scale, axis=1) @ v + u
  learn_item = softmax((v @ u.T) * VU_adj * scale, axis=1) @ u + v

Sharding: core i owns rows [i*1024, (i+1)*1024) of BOTH outputs; no
collectives (the contracted-side projection is replicated).

v4 design (fp8 DoubleRow, deep-pipelined):
- All big matmuls (scores, aggregation, denominator, projections) run in
  fp8e4 with perf_mode=DoubleRow (2 k-chunks per instruction, ~1.5x PE).
- Feature matrices are projected twice: fT [h, N] (feature-major, biased,
  used as score lhsT) and vrow [N, h] (row-major, UNbiased, used as
  aggregation rhs). The missing bias in vrow cancels through softmax:
  P@(v + 1 b^T)/rsum = P@vrow/rsum + b^T, so b_feat is folded into the
  residual qrow instead. This removes all per-block PE transposes.
- Per-core inputs are column-ROLLED so this core's rows are columns
  [0:RB) of both feature matrices; the score rhs (qTb) is then just
  fT_other[:, :, 0:RB] - no separate query projection.
- exp uses bias -ln(32): softmax is shift-invariant, masked entries
  become exactly 1/32 (fp8-exact), max value ~5 stays far below fp8e4
  max 240.
- Residual path stays accurate: qrow = f32r projection of the f32 query
  rows -> bf16, + (b_q + b_feat) broadcast row.
- v4 scheduling: fp8 projections run FIRST (first matmul only needs
  ~0.5MB of DMA, not 3.5MB); qrow streams per-column-chunk; PSUM is
  organized as 8 one-bank tiles so projections get a 7-deep ring; the
  attention epilogue + final aggregation of each row-block are deferred
  into the next block's score loop so the PE never drains at block
  boundaries.
"""

import sys

sys.path.insert(0, "/opt/trn_rl_repo")

import ml_dtypes
import numpy as np

import concourse.bacc as bacc
import concourse.bass as bass
import concourse.mybir as mybir
import concourse.tile as tile
from concourse.bass_utils import run_bass_kernel_spmd

N = 8192          # users == items
H = 512           # hidden
NCORES = 8
RB = N // NCORES  # 1024 rows per core per direction
KH = H // 128     # 4 h-chunks
NB = N // 128     # 64 column chunks
NBP = NB // 2     # 32 column-pair chunks (DoubleRow)
NRB = RB // 512   # 2 r-blocks of 512
NJ = N // 512     # 16 512-col blocks for projection streaming
SCALE = float(1.0 / np.sqrt(np.float32(H)))
NLN32 = float(-np.log(32.0))

F32 = mybir.dt.float32
F32R = mybir.dt.float32r
BF16 = mybir.dt.bfloat16
FP8 = mybir.dt.float8e4
NP_FP8 = ml_dtypes.float8_e4m3
DR = mybir.MatmulPerfMode.DoubleRow


def build_nc():
    nc = bacc.Bacc("TRN2", target_bir_lowering=False, debug=False)

    featA = nc.declare_dram_parameter("featA", [H, N], FP8, isOutput=False)
    featB = nc.declare_dram_parameter("featB", [H, N], FP8, isOutput=False)
    qtA = nc.declare_dram_parameter("qtA", [4, 128, KH * 256], F32,
                                    isOutput=False)
    qtB = nc.declare_dram_parameter("qtB", [4, 128, KH * 256], F32,
                                    isOutput=False)
    maskA = nc.declare_dram_parameter("maskA", [NB, 128, RB], FP8, isOutput=False)
    maskB = nc.declare_dram_parameter("maskB", [NB, 128, RB], FP8, isOutput=False)
    WfA = nc.declare_dram_parameter("WfA", [128, KH, H], FP8, isOutput=False)
    WfB = nc.declare_dram_parameter("WfB", [128, KH, H], FP8, isOutput=False)
    WqA = nc.declare_dram_parameter("WqA", [H, H], F32, isOutput=False)
    WqB = nc.declare_dram_parameter("WqB", [H, H], F32, isOutput=False)
    bfA = nc.declare_dram_parameter("bfA", [128, KH], F32, isOutput=False)
    bfB = nc.declare_dram_parameter("bfB", [128, KH], F32, isOutput=False)
    brow = nc.declare_dram_parameter("brow", [128, H], F32, isOutput=False)
    out = nc.declare_dram_parameter("out", [2 * RB, H], F32, isOutput=True)

    with tile.TileContext(nc) as tc:
        with (
            tc.tile_pool(name="bigA", bufs=1) as bigA,
            tc.tile_pool(name="bigB", bufs=1) as bigB,
            tc.tile_pool(name="wts", bufs=1) as wts,
            tc.tile_pool(name="wtsB", bufs=1) as wtsB,
            tc.tile_pool(name="stream", bufs=6) as stream,
            tc.tile_pool(name="qc", bufs=2) as qcp,
            tc.tile_pool(name="mask", bufs=3) as maskp,
            tc.tile_pool(name="pf", bufs=3) as pfp,
            tc.tile_pool(name="pb", bufs=3) as pbp,
            tc.tile_pool(name="outs", bufs=1) as outsp,
            tc.tile_pool(name="small", bufs=1) as small,
            # 8 PSUM banks as single-bank tiles: ps_a holds the 4 agg
            # accumulators during attention (and joins the projection
            # ring before that), ps_b holds score tiles, ps_rs the
            # denominator.
            tc.tile_pool(name="ps_a", bufs=4, space="PSUM") as ps_a,
            tc.tile_pool(name="ps_b", bufs=3, space="PSUM") as ps_b,
            tc.tile_pool(name="ps_rs", bufs=1, space="PSUM") as ps_rs,
        ):
            ones2 = small.tile([128, 2, 16], FP8, tag="ones")
            nc.vector.memset(ones2[:], 1.0)
            nbias = small.tile([128, 1], F32, tag="nbias")
            nc.vector.memset(nbias[:], NLN32)
            brow_sb = small.tile([128, H], F32, tag="brow")
            bfA_sb = small.tile([128, KH], F32, tag="bfA")
            bfB_sb = small.tile([128, KH], F32, tag="bfB")

            # persistent per-direction tensors
            fT = {}
            vrow = {}
            qrow = {}
            for big_pool, d in ((bigA, "A"), (bigB, "B")):
                fT[d] = big_pool.tile([128, KH, N], FP8, tag=f"fT{d}",
                                      name=f"fT{d}")
                vrow[d] = big_pool.tile([128, NB, H], FP8, tag=f"vrow{d}",
                                        name=f"vrow{d}")
                qrow[d] = big_pool.tile([128, 2 * KH, H], BF16, tag=f"qrow{d}",
                                        name=f"qrow{d}")

            # 7-deep psum ring for the projection phase (attention pins
            # ps_a's 4 bufs as accumulators, projections may rotate
            # through everything).
            _ring = [0]

            def proj_ps():
                _ring[0] += 1
                pool = (ps_a, ps_b)[_ring[0] % 2]
                ps = pool.tile([128, 512], F32, tag="s",
                               name=f"ps{_ring[0]}")
                return ps

            # ---------------- phase 0a: fp8 projections ----------------
            def project_fp8(d, feat_dram, wf_dram, bias_f, side_tasks=(),
                            first_hook=None):
                side_tasks = list(side_tasks)
                wfp = wts.tile([128, KH, H], FP8, tag=f"wfp{d}",
                               name=f"wfp{d}")
                nc.sync.dma_start(wfp[:], wf_dram[:])
                for j in range(NJ):
                    ft_in = stream.tile([128, KH, 512], FP8, tag="ft",
                                        name=f"ft{d}{j}")
                    for k in range(KH):
                        nc.sync.dma_start(
                            ft_in[:, k, :],
                            feat_dram[k * 128:(k + 1) * 128,
                                      j * 512:(j + 1) * 512])
                        if j == 0 and k == 1 and first_hook is not None:
                            # slot small non-gating loads behind the two
                            # chunks the first matmul actually waits on
                            first_hook()
                    for m in range(KH):
                        ps = proj_ps()
                        for ko in range(2):
                            nc.tensor.matmul(
                                ps[:],
                                wfp[:, 2 * ko:2 * ko + 2, m * 128:(m + 1) * 128],
                                ft_in[:, 2 * ko:2 * ko + 2, :],
                                start=(ko == 0), stop=(ko == 1), perf_mode=DR)
                        if m % 2 == 0:
                            nc.vector.tensor_scalar(
                                out=fT[d][:, m, j * 512:(j + 1) * 512],
                                in0=ps[:], scalar1=bias_f[:, m:m + 1],
                                scalar2=None, op0=mybir.AluOpType.add)
                        else:
                            nc.scalar.add(
                                fT[d][:, m, j * 512:(j + 1) * 512], ps[:],
                                bias_f[:, m:m + 1])
                    for sub in range(4):
                        c = j * 4 + sub
                        ps = proj_ps()
                        for ko in range(2):
                            nc.tensor.matmul(
                                ps[:],
                                ft_in[:, 2 * ko:2 * ko + 2,
                                      sub * 128:(sub + 1) * 128],
                                wfp[:, 2 * ko:2 * ko + 2, :],
                                start=(ko == 0), stop=(ko == 1), perf_mode=DR)
                        if sub % 2 == 0:
                            nc.vector.tensor_copy(vrow[d][:, c, :], ps[:])
                        else:
                            nc.scalar.copy(vrow[d][:, c, :], ps[:])
                    # one interleaved residual-projection task per j-block:
                    # fills the DMA-gated moments of this stream with the
                    # qrow f32r matmuls, eliminating a separate qrow phase.
                    if side_tasks:
                        side_tasks.pop(0)()

            # ---------------- phase 0b: residual projections ----------------
            # wq for A lives in the wts pool; wq for B reuses the (now
            # idle) fp8 feature-stream pool slots, so its DMA can land
            # while qrow A computes.
            def load_wq(d, wq_dram, pool):
                wq = [pool.tile([128, H], F32R, tag=f"wq{k}", name=f"wq{d}{k}")
                      for k in range(KH)]
                for k in range(KH):
                    nc.sync.dma_start(
                        wq[k][:], wq_dram[k * 128:(k + 1) * 128, :].bitcast(F32R))
                return wq

            def qrow_task(d, qt_dram, c, state):
                def task():
                    c2, cc = divmod(c, 2)
                    if cc == 0:
                        qc = qcp.tile([128, KH, 256], F32R, tag="qt",
                                      name=f"qt{d}{c2}")
                        nc.sync.dma_start(
                            qc[:], qt_dram[c2].bitcast(F32R).rearrange(
                                "p (k c) -> p k c", k=KH))
                        state["qc"] = qc
                    qc = state["qc"]
                    wq = wq_sb[d]
                    ps = proj_ps()
                    for k in range(KH):
                        nc.tensor.matmul(
                            ps[:], qc[:, k, cc * 128:(cc + 1) * 128],
                            wq[k][:],
                            start=(k == 0), stop=(k == KH - 1))
                    nc.vector.tensor_tensor(
                        out=qrow[d][:, c, :], in0=ps[:], in1=brow_sb[:],
                        op=mybir.AluOpType.add)
                return task

            project_fp8("A", featA, WfA, bfA_sb,
                        first_hook=lambda: nc.sync.dma_start(
                            bfA_sb[:], bfA[:]))
            # wq DMAs hoisted here: they execute behind projection A's
            # feature stream, well before the qrow tasks need them.
            wq_sb = {"A": load_wq("A", WqA, wts), "B": load_wq("B", WqB, wtsB)}
            # brow (first used by the qrow adds, t~75us) and bfB (first
            # used by projection B, t~73us) dispatch here, off the startup
            # critical path: they execute behind projection A's stream.
            nc.sync.dma_start(brow_sb[:], brow[:])
            nc.sync.dma_start(bfB_sb[:], bfB[:])
            _qstate = {}
            project_fp8("B", featB, WfB, bfB_sb, side_tasks=(
                [qrow_task("A", qtA, c, _qstate) for c in range(2 * KH)]
                + [qrow_task("B", qtB, c, _qstate) for c in range(2 * KH)]))

            # ---------------- attention ----------------
            # Flat schedule over 4 row-blocks (A rb0, A rb1, B rb0, B rb1).
            # The final aggregation (pend) and the epilogue (prev) of each
            # block are emitted inside the NEXT block's score loop, so the
            # PE keeps streaming across boundaries.

            def emit_agg(agg4, rsum4, pbf2, bp, myvrow, rsum_first=False):
                if rsum_first:
                    # final flush only: the epilogue's recip waits on the
                    # rsum stop, so complete the denominator before the
                    # agg matmuls (steady-state order stays interleaved -
                    # reordering it regresses PE pacing).
                    for rs in range(4):
                        nc.tensor.matmul(
                            rsum4[:, rs:rs + 1],
                            pbf2[:, :, rs * 128:(rs + 1) * 128],
                            ones2[:, :, 0:1],
                            start=(bp == 0 and rs == 0),
                            stop=(bp == NBP - 1 and rs == 3), perf_mode=DR)
                    for rs in range(4):
                        nc.tensor.matmul(
                            agg4[rs][:], pbf2[:, :, rs * 128:(rs + 1) * 128],
                            myvrow[:, 2 * bp:2 * bp + 2, :],
                            start=(bp == 0), stop=(bp == NBP - 1),
                            perf_mode=DR)
                    return
                for rs in range(4):
                    nc.tensor.matmul(
                        agg4[rs][:], pbf2[:, :, rs * 128:(rs + 1) * 128],
                        myvrow[:, 2 * bp:2 * bp + 2, :],
                        start=(bp == 0), stop=(bp == NBP - 1), perf_mode=DR)
                    # all 4 columns form ONE psum accumulation group (they
                    # share a 2KB zero region): start only on the very first
                    # matmul, stop only on the very last
                    nc.tensor.matmul(
                        rsum4[:, rs:rs + 1],
                        pbf2[:, :, rs * 128:(rs + 1) * 128],
                        ones2[:, :, 0:1],
                        start=(bp == 0 and rs == 0),
                        stop=(bp == NBP - 1 and rs == 3), perf_mode=DR)

            def emit_epilogue(d, rb, agg4, rsum4, out_base, final=False):
                recip = small.tile([128, 4], F32, tag="recip")
                nc.vector.reciprocal(recip[:], rsum4[:])
                for rs in range(4):
                    o_sb = outsp.tile([128, H], F32, tag=f"o{rs}",
                                      name=f"o{d}{rb}_{rs}")
                    if rs % 2 == 0:
                        nc.vector.tensor_scalar(
                            out=o_sb[:], in0=agg4[rs][:],
                            scalar1=recip[:, rs:rs + 1], scalar2=None,
                            op0=mybir.AluOpType.mult)
                    else:
                        nc.scalar.mul(o_sb[:], agg4[rs][:],
                                      recip[:, rs:rs + 1])
                    # the +qrow adds touch only SBUF, so they go on the
                    # idle GpSimd (mid-kernel: keeps DVE/Scalar clear for
                    # the PSUM reads the next block's agg WAR-waits on);
                    # the FINAL epilogue is a serial tail, so split its
                    # adds with DVE (GpSimd adds are ~1.6us each).
                    add_eng = nc.vector if (final and rs < 3) \
                        else nc.gpsimd
                    add_eng.tensor_tensor(
                        out=o_sb[:], in0=o_sb[:],
                        in1=qrow[d][:, rb * 4 + rs, :],
                        op=mybir.AluOpType.add)
                    row0 = out_base + rb * 512 + rs * 128
                    # final epilogue: spread the four out-DMAs across four
                    # engine queues so the 256KB transfers run in parallel
                    # instead of serializing behind one dispatch queue -
                    # the end-of-kernel barrier waits on the last packet.
                    # mid-kernel: out-DMAs ride the GpSimd queue (it just
                    # did the adds and then idles) so the sync queue stays
                    # clear for the next block's mask stream at boundaries
                    dma_eng = (nc.sync, nc.scalar, nc.gpsimd,
                               nc.scalar)[rs] if final else nc.gpsimd
                    dma_eng.dma_start(out[row0:row0 + 128, :], o_sb[:])

            blocks = [(d, other, mask_dram, out_base, rb)
                      for (d, other, mask_dram, out_base) in
                      (("A", "B", maskA, 0), ("B", "A", maskB, RB))
                      for rb in range(NRB)]

            pendq = []   # (agg4, rsum4, pbf2, bp, vrow) awaiting aggregation
            prev = None  # (d, rb, agg4, rsum4, out_base) awaiting epilogue
            for bi, (d, other, mask_dram, out_base, rb) in enumerate(blocks):
                myfT = fT[d]
                qTb = fT[other]
                boundary = bi > 0
                agg4 = [ps_a.tile([128, 512], F32, tag="s",
                                  name=f"agg{d}{rb}_{rs}") for rs in range(4)]
                rsum4 = ps_rs.tile([128, 4], F32, tag="rs")
                for bp in range(NBP):
                    sps = []
                    for t in range(2):
                        b = 2 * bp + t
                        sp = ps_b.tile([128, 512], F32, tag="s")
                        for ko in range(2):
                            nc.tensor.matmul(
                                sp[:],
                                myfT[:, 2 * ko:2 * ko + 2,
                                     b * 128:(b + 1) * 128],
                                qTb[:, 2 * ko:2 * ko + 2,
                                    rb * 512:(rb + 1) * 512],
                                start=(ko == 0), stop=(ko == 1),
                                perf_mode=DR)
                        sps.append(sp)

                    # aggregate earlier pairs while DVE/Act chew on this
                    # one. At a block boundary: flush the old block's last
                    # aggregation + epilogue at bp0, then hold this
                    # block's first aggregations until bp2 so the new
                    # PSUM accumulation group doesn't wait on the old
                    # epilogue's reads.
                    if pendq:
                        emit_agg(*pendq.pop(0))
                    if prev is not None:
                        emit_epilogue(*prev)
                        prev = None
                    mt = maskp.tile([128, 2, 512], FP8, tag="mk")
                    nc.sync.dma_start(
                        mt[:],
                        mask_dram[2 * bp:2 * bp + 2, :,
                                  rb * 512:(rb + 1) * 512].rearrange(
                                      "t p c -> p t c"))
                    pbf2 = pbp.tile([128, 2, 512], FP8, tag="pbf")
                    for t in range(2):
                        # gpsimd cannot read PSUM; both mults go on DVE
                        p32 = pfp.tile([128, 512], F32, tag="p32")
                        nc.vector.tensor_tensor(
                            out=p32[:], in0=sps[t][:], in1=mt[:, t, :],
                            op=mybir.AluOpType.mult)
                        nc.scalar.activation(
                            pbf2[:, t, :], p32[:],
                            mybir.ActivationFunctionType.Exp,
                            bias=nbias[:], scale=SCALE)
                    pendq.append((agg4, rsum4, pbf2, bp, vrow[d]))
                prev = (d, rb, agg4, rsum4, out_base)
            while pendq:
                emit_agg(*pendq.pop(0), rsum_first=True)
            emit_epilogue(*prev, final=True)

    nc.compile()
    return nc


_NC_CACHE = None
TRACE = False
LAST_RESULT = None


def kernel(user, item, UV_adj, VU_adj, W_u, b_u, W_v, b_v):
    global _NC_CACHE, LAST_RESULT
    user = np.asarray(user, dtype=np.float32)
    item = np.asarray(item, dtype=np.float32)
    UV_adj = np.asarray(UV_adj, dtype=np.float32)
    VU_adj = np.asarray(VU_adj, dtype=np.float32)
    W_u = np.asarray(W_u, dtype=np.float32)
    W_v = np.asarray(W_v, dtype=np.float32)
    b_u = np.asarray(b_u, dtype=np.float32)
    b_v = np.asarray(b_v, dtype=np.float32)

    userT = np.ascontiguousarray(user.T)
    itemT = np.ascontiguousarray(item.T)
    userT8 = userT.astype(NP_FP8)
    itemT8 = itemT.astype(NP_FP8)
    UV8 = UV_adj.astype(NP_FP8)
    VU8 = np.ascontiguousarray(UV8.T)
    W_uT = np.ascontiguousarray(W_u.T)
    W_vT = np.ascontiguousarray(W_v.T)
    # [128, KH, H] fp8 weight layout for DoubleRow projections
    WfA_np = np.ascontiguousarray(
        W_vT.reshape(KH, 128, H).transpose(1, 0, 2).astype(NP_FP8))
    WfB_np = np.ascontiguousarray(
        W_uT.reshape(KH, 128, H).transpose(1, 0, 2).astype(NP_FP8))
    bfA_np = np.ascontiguousarray(b_v.reshape(KH, 128).T)
    bfB_np = np.ascontiguousarray(b_u.reshape(KH, 128).T)
    brow_np = np.ascontiguousarray(
        np.broadcast_to((b_u + b_v)[None, :], (128, H)))

    in_maps = []
    for i in range(NCORES):
        r = i * RB
        sl = slice(r, r + RB)
        in_maps.append({
            # feature matrices with this core's rows rolled to the front
            "featA": np.ascontiguousarray(np.roll(itemT8, -r, axis=1)),
            "featB": np.ascontiguousarray(np.roll(userT8, -r, axis=1)),
            "qtA": np.ascontiguousarray(
                userT[:, sl].reshape(KH, 128, 4, 256).transpose(2, 1, 0, 3)
                .reshape(4, 128, KH * 256)),
            "qtB": np.ascontiguousarray(
                itemT[:, sl].reshape(KH, 128, 4, 256).transpose(2, 1, 0, 3)
                .reshape(4, 128, KH * 256)),
            "maskA": np.ascontiguousarray(
                np.roll(VU8[:, sl], -r, axis=0)).reshape(NB, 128, RB),
            "maskB": np.ascontiguousarray(
                np.roll(UV8[:, sl], -r, axis=0)).reshape(NB, 128, RB),
            "WfA": WfA_np,
            "WfB": WfB_np,
            "WqA": W_uT,
            "WqB": W_vT,
            "bfA": bfA_np,
            "bfB": bfB_np,
            "brow": brow_np,
        })

    if _NC_CACHE is None:
        _NC_CACHE = build_nc()
    res = run_bass_kernel_spmd(_NC_CACHE, in_maps, core_ids=list(range(NCORES)),
                               trace=TRACE)
    LAST_RESULT = res
    results = res.results
    learn_user = np.concatenate([results[i]["out"][:RB] for i in range(NCORES)], 0)
    learn_item = np.concatenate([results[i]["out"][RB:] for i in range(NCORES)], 0)
    return (learn_user, learn_item)


if __name__ == "__main__":
    nc = build_nc()
    print("built ok")
